# revision 7
# baseline (speedup 1.0000x reference)
"""APGI network Bass kernel for 8 TRN2 NeuronCores (pure data parallel).

Layout: feature-major (batch on the free/column axis). Host pre-transposes
inputs per core; device computes all 9 outputs feature-major; host
transposes back.

Self-contained: hardcodes shapes from the problem spec (B=524288, E=128,
I=32, C=8, A=16), 8 cores.
"""
import os
import sys
import numpy as np

sys.path.insert(0, "/opt/trn_rl_repo")

import concourse.bass as bass
import concourse.bacc as bacc
import concourse.tile as tile
from concourse import mybir
from concourse.bass_utils import run_bass_kernel_spmd

AF = mybir.ActivationFunctionType
ALU = mybir.AluOpType
f32 = mybir.dt.float32
f32r = mybir.dt.float32r
bf16 = mybir.dt.bfloat16

B = 524288
N_CORES = 8
B_LOC = B // N_CORES

# Column tiling: F batch columns per iteration, split into 4 chunks of C.
F = 2048
C = F // 4


# ---------------------------------------------------------------------------
# Host-side weight packing
# ---------------------------------------------------------------------------
def _rep4(vals, offs, width=1):
    """Replicate a per-row pattern into all four 32-row blocks.

    vals: [n, width]; placed at rows 32g+offs .. 32g+offs+n for g in 0..3.
    """
    out = np.zeros((128, width), np.float32)
    v = np.asarray(vals, np.float32).reshape(-1, width)
    n = v.shape[0]
    for g in range(4):
        out[32 * g + offs:32 * g + offs + n, :] = v
    return out


def prep_weights(p):
    """Pack params dict into wR (f32r lhsT blob), wB (bf16 lhsT blob),
    bias blob, plus slice metadata. All np.float32 host-side."""
    P = {k: np.asarray(v, np.float32) for k, v in p.items()}
    alpha = float(abs(np.float32(P["alpha"])))
    beta = float(abs(np.float32(P["beta"])))

    # --- f32r blob: e1 and i1th1 (input layers) ---
    wR = np.zeros((128, 128 + 80), np.float32)
    wR[0:128, 0:128] = P["ew1"].T                     # e1 lhsT [128,128]
    wR[0:32, 128:192] = P["iw1"].T                    # i1 [32,64]
    wR[32:40, 192:208] = P["th_w1"].T                 # th1 [8,16]
    rs = {"e1": (slice(0, 128), slice(0, 128)),
          "i1th1": (slice(0, 40), slice(128, 208))}

    # --- bf16 blob ---
    cols = []
    bs = {}

    def add(name, rows, mat):
        off = sum(c.shape[1] for c in cols)
        m = np.zeros((128, mat.shape[1]), np.float32)
        m[rows, :] = mat
        cols.append(m)
        bs[name] = (rows, slice(off, off + mat.shape[1]))

    add("e2", slice(0, 128), P["ew2"].T)              # [128,64]
    add("i2", slice(0, 64), P["iw2"].T)               # [64,32]
    m = np.zeros((96, 48), np.float32)
    m[0:64, 0:32] = P["ew3"].T
    m[64:96, 32:48] = P["iw3"].T
    add("e3i3", slice(0, 96), m)
    m = np.zeros((48, 88), np.float32)
    m[:, 0:64] = P["ws_w"].T
    m[0:32, 64:80] = P["pe_w1"].T
    m[32:48, 80:88] = P["pi_w1"].T
    add("wspepi", slice(0, 48), m)
    m = np.zeros((64, 96), np.float32)
    m[:, 0:64] = P["so_w1"].T
    m[:, 64:96] = P["va_w1"].T
    add("so1va1", slice(0, 64), m)
    add("so2", slice(0, 64), P["so_w2"].T)
    add("po1", slice(0, 64), P["po_w1"].T)
    m = np.zeros((64, 33), np.float32)
    m[0:32, 0:16] = P["po_w2"].T
    m[32:64, 32:33] = P["va_w2"].T
    add("po2va2", slice(0, 64), m)
    # packed lhsTs
    m = np.zeros((48, 2), np.float32)
    m[0:32, 0] = 1.0
    m[32:48, 1] = 1.0
    add("epsones", slice(0, 48), m)                   # sum-sq reduce
    m = np.zeros((24, 2), np.float32)
    m[0:16, 0:1] = P["pe_w2"].T
    m[16:24, 1:2] = P["pi_w2"].T
    add("pepi2", slice(0, 128), _repmat(m, 24))
    add("th2", slice(0, 128), _repmat(P["th_w2"].T, 16))
    wih = P["gru_wih"]                                # [48,2]
    add("gir", slice(0, 128), _repmat(wih[0:16].T, 2))
    add("giz", slice(0, 128), _repmat(wih[16:32].T, 2))
    add("gin", slice(0, 128), _repmat(wih[32:48].T, 2))
    add("ones16", slice(0, 128), _repmat(np.ones((16, 1), np.float32), 16))
    add("ones64", slice(0, 128), np.ones((128, 64), np.float32))
    wB = np.concatenate(cols, axis=1)

    # --- bias blob (f32, one [128,1] column per entry) ---
    bcols = []
    bbs = {}

    def addb(name, col):
        bbs[name] = (sum(c.shape[1] for c in bcols),)
        bcols.append(np.asarray(col, np.float32).reshape(128, 1))

    def at(rows, vals):
        c = np.zeros((128, 1), np.float32)
        c[rows, 0] = vals
        return c

    addb("e1", at(slice(0, 128), P["eb1"]))
    c2 = np.zeros((128, 1), np.float32)
    c2[0:64, 0] = P["ib1"]
    c2[64:80, 0] = P["th_b1"]
    addb("i1th1", c2)
    addb("e2", at(slice(0, 64), P["eb2"]))
    addb("i2", at(slice(0, 32), P["ib2"]))
    c2 = np.zeros((128, 1), np.float32)
    c2[0:32, 0] = P["eb3"]
    c2[32:48, 0] = P["ib3"]
    addb("c48", c2)
    c2 = np.zeros((128, 1), np.float32)
    c2[0:64, 0] = P["ws_b"]
    c2[64:80, 0] = P["pe_b1"]
    c2[80:88, 0] = P["pi_b1"]
    addb("wspepi", c2)
    c2 = np.zeros((128, 1), np.float32)
    c2[0:64, 0] = P["so_b1"]
    c2[64:96, 0] = P["va_b1"]
    addb("so1va1", c2)
    addb("so2", at(slice(0, 64), P["so_b2"]))
    addb("so2n", at(slice(0, 64), -P["so_b2"]))
    addb("po1", at(slice(0, 32), P["po_b1"]))
    addb("po2", at(slice(0, 16), P["po_b2"]))
    addb("va2", at(slice(32, 33), P["va_b2"]))
    addb("eps", _rep4([[0.0], [np.log(beta)]], 0))
    addb("pi", _rep4([[P["pe_b2"][0]], [P["pi_b2"][0]]], 0))
    addb("thn", _rep4([[-P["th_b2"][0]]], 0))
    bih, bhh = P["gru_bih"], P["gru_bhh"]
    addb("rn", _rep4(-(bih[0:16] + bhh[0:16]).reshape(-1, 1), 0))
    addb("z", _rep4((bih[16:32] + bhh[16:32]).reshape(-1, 1), 0))
    addb("n2", _rep4((2.0 * bih[32:48]).reshape(-1, 1), 0))
    addb("bhhn", _rep4(bhh[32:48].reshape(-1, 1), 0))
    bias = np.concatenate(bcols, axis=1)

    return wR, rs, wB, bs, bias, bbs, alpha


def _repmat(m, nrows):
    """Replicate [nrows, w] matrix into rows 32g..32g+nrows for g=0..3."""
    out = np.zeros((128, m.shape[1]), np.float32)
    for g in range(4):
        out[32 * g:32 * g + nrows, :] = m
    return out


# ---------------------------------------------------------------------------
# Device kernel builder
# ---------------------------------------------------------------------------
def build(nc, b_loc, wR_cols, wB_cols, bias_cols, alpha):
    nit = b_loc // F

    xe_ext = nc.declare_dram_parameter("xe", [128, b_loc], f32, isOutput=False)
    xic_ext = nc.declare_dram_parameter("xic", [40, b_loc], f32, isOutput=False)
    wR_ext = nc.declare_dram_parameter("wR", [128, wR_cols], f32, isOutput=False)
    wB_ext = nc.declare_dram_parameter("wB", [128, wB_cols], f32, isOutput=False)
    bb_ext = nc.declare_dram_parameter("bb", [128, bias_cols], f32, isOutput=False)

    o_pol = nc.declare_dram_parameter("pol", [16, b_loc], f32, isOutput=True)
    o_val = nc.declare_dram_parameter("val", [1, b_loc], f32, isOutput=True)
    o_ign = nc.declare_dram_parameter("ign", [1, b_loc], f32, isOutput=True)
    o_s = nc.declare_dram_parameter("st", [1, b_loc], f32, isOutput=True)
    o_th = nc.declare_dram_parameter("th", [1, b_loc], f32, isOutput=True)
    o_pi = nc.declare_dram_parameter("pi2", [2, b_loc], f32, isOutput=True)
    o_som = nc.declare_dram_parameter("som", [64, b_loc], f32, isOutput=True)
    o_gat = nc.declare_dram_parameter("gat", [64, b_loc], f32, isOutput=True)

    return (xe_ext, xic_ext, wR_ext, wB_ext, bb_ext,
            o_pol, o_val, o_ign, o_s, o_th, o_pi, o_som, o_gat), nit


def emit(nc, tc, ext, nit, rs, bs, bbs, alpha, wR_cols, wB_cols, bias_cols):
    (xe_ext, xic_ext, wR_ext, wB_ext, bb_ext,
     o_pol, o_val, o_ign, o_s, o_th, o_pi, o_som, o_gat) = ext

    import contextlib
    ctx = contextlib.ExitStack()
    wpool = ctx.enter_context(tc.tile_pool(name="w", bufs=1))
    inp = ctx.enter_context(tc.tile_pool(name="inp", bufs=2))
    act = ctx.enter_context(tc.tile_pool(name="act", bufs=1))
    tl = ctx.enter_context(tc.tile_pool(name="tl", bufs=1))
    psw = ctx.enter_context(tc.tile_pool(name="psw", bufs=3, space="PSUM"))
    psp = ctx.enter_context(tc.tile_pool(name="psp", bufs=4, space="PSUM"))

    # --- weights / biases, loaded once ---
    wR = wpool.tile([128, wR_cols], f32r, name="wR")
    nc.gpsimd.dma_start(wR[:], wR_ext[:])
    wBt = wpool.tile([128, wB_cols], bf16, name="wBt")
    nc.gpsimd.dma_start(wBt[:], wB_ext[:])
    bb = wpool.tile([128, bias_cols], f32, name="bb")
    nc.sync.dma_start(bb[:], bb_ext[:])

    def W(name):
        r, c = bs[name]
        return wBt[r, c]

    def Bc(name, rows=slice(0, 128)):
        return bb[rows, bbs[name][0]:bbs[name][0] + 1]

    for it in range(nit):
        col0 = it * F
        xe = inp.tile([128, F], f32r, name="xe")
        nc.gpsimd.dma_start(xe[:], xe_ext[:, col0:col0 + F])
        xic = inp.tile([40, F], f32r, name="xic")
        nc.gpsimd.dma_start(xic[:], xic_ext[:, col0:col0 + F])

        h1e = act.tile([128, F], bf16, name="h1e")
        h1i = act.tile([64, F], bf16, name="h1i")
        th1h = tl.tile([128, C], bf16, name="th1h")
        h2 = act.tile([96, F], bf16, name="h2")
        c48 = act.tile([48, F], bf16, name="c48")
        c48sq = act.tile([48, F], bf16, name="c48sq")
        ws = act.tile([64, F], f32, name="ws")
        pepih = tl.tile([128, C], bf16, name="pepih")

        p_eps = psp.tile([128, C], f32, name="p_eps", tag="pp")
        p_pi = psp.tile([128, C], f32, name="p_pi", tag="pp")
        p_th = psp.tile([128, C], f32, name="p_th", tag="pp")

        for g in range(4):
            cs = slice(g * C, (g + 1) * C)
            rg = 32 * g
            # ---- encoders ----
            p1 = psw.tile([128, C], f32, name="p1", tag="pw")
            nc.tensor.matmul(p1[:], wR[rs["e1"][0], rs["e1"][1]], xe[:, cs],
                             start=True, stop=True)
            nc.scalar.activation(h1e[:, cs], p1[:], AF.Relu, bias=Bc("e1"))

            p2 = psw.tile([80, C], f32, name="p2", tag="pw")
            nc.tensor.matmul(p2[:], wR[rs["i1th1"][0], rs["i1th1"][1]],
                             xic[:, cs], start=True, stop=True)
            nc.scalar.activation(h1i[:, cs], p2[0:64, :], AF.Relu,
                                 bias=Bc("i1th1", slice(0, 64)))
            nc.scalar.activation(th1h[rg:rg + 16, :], p2[64:80, :], AF.Relu,
                                 bias=Bc("i1th1", slice(64, 80)))

            p3 = psw.tile([64, C], f32, name="p3", tag="pw")
            nc.tensor.matmul(p3[:], W("e2"), h1e[:, cs], start=True, stop=True)
            nc.vector.tensor_scalar(h2[0:64, cs], p3[:], Bc("e2", slice(0, 64)),
                                    0.0, ALU.add, ALU.max)

            p4 = psw.tile([32, C], f32, name="p4", tag="pw")
            nc.tensor.matmul(p4[:], W("i2"), h1i[:, cs], start=True, stop=True)
            nc.vector.tensor_scalar(h2[64:96, cs], p4[:], Bc("i2", slice(0, 32)),
                                    0.0, ALU.add, ALU.max)

            p5 = psw.tile([48, C], f32, name="p5", tag="pw")
            nc.tensor.matmul(p5[:], W("e3i3"), h2[:, cs], start=True, stop=True)
            nc.scalar.activation(c48[:, cs], p5[:], AF.Identity, bias=Bc("c48", slice(0, 48)))
            nc.scalar.activation(c48sq[:, cs], p5[:], AF.Square, bias=Bc("c48", slice(0, 48)))

            p6 = psw.tile([88, C], f32, name="p6", tag="pw")
            nc.tensor.matmul(p6[:], W("wspepi"), c48[:, cs], start=True, stop=True)
            nc.vector.tensor_scalar(ws[:, cs], p6[0:64, :],
                                    Bc("wspepi", slice(0, 64)), None, ALU.add)
            nc.vector.tensor_scalar(pepih[rg:rg + 24, :], p6[64:88, :],
                                    Bc("wspepi", slice(64, 88)), 0.0, ALU.add, ALU.max)

            # ---- packed reductions (bf16, tile_position) ----
            nc.tensor.matmul(p_eps[rg:rg + 2, :], W("epsones")[0:48, :],
                             c48sq[:, cs], start=True, stop=True,
                             tile_position=(0, rg))
            nc.tensor.matmul(p_pi[rg:rg + 2, :], W("pepi2")[rg:rg + 24, :],
                             pepih[rg:rg + 24, :], start=True, stop=True,
                             tile_position=(rg, rg))
            nc.tensor.matmul(p_th[rg:rg + 1, :], W("th2")[rg:rg + 16, :],
                             th1h[rg:rg + 16, :], start=True, stop=True,
                             tile_position=(rg, rg))

        # ---- narrow tail, packed x4 [128, C] ----
        t_ln = tl.tile([128, C], f32, name="t_ln", tag="sc", bufs=4)
        nc.scalar.activation(t_ln[:], p_eps[:], AF.Ln)
        t_eps = tl.tile([128, C], f32, name="t_eps")
        nc.scalar.activation(t_eps[:], t_ln[:], AF.Exp, bias=Bc("eps"), scale=0.5)

        t_pe = tl.tile([128, C], f32, name="t_pe", tag="sc", bufs=4)
        nc.scalar.activation(t_pe[:], p_pi[:], AF.Exp, bias=Bc("pi"))
        t_pi = tl.tile([128, C], f32, name="t_pi")
        nc.scalar.activation(t_pi[:], t_pe[:], AF.Ln, bias=1.0)

        t_sur = tl.tile([128, C], bf16, name="t_sur")
        nc.vector.tensor_mul(t_sur[:], t_eps[:], t_pi[:])

        # theta = sigmoid(th2+b) = recip(1+exp(-x-b))
        t_te = tl.tile([128, C], f32, name="t_te", tag="sc", bufs=4)
        nc.scalar.activation(t_te[:], p_th[:], AF.Exp, bias=Bc("thn"), scale=-1.0)
        nc.vector.tensor_scalar(t_te[:], t_te[:], 1.0, None, ALU.add)
        t_th = tl.tile([128, C], f32, name="t_th")
        nc.vector.reciprocal_approx_fast(t_th[:], t_te[:])

        # GRU gates
        p_r = psp.tile([128, C], f32, name="p_r", tag="pp")
        p_z = psp.tile([128, C], f32, name="p_z", tag="pp")
        p_n = psp.tile([128, C], f32, name="p_n", tag="pp")
        for g in range(4):
            rg = 32 * g
            nc.tensor.matmul(p_r[rg:rg + 16, :], W("gir")[rg:rg + 2, :],
                             t_sur[rg:rg + 2, :], start=True, stop=True,
                             tile_position=(rg, rg))
            nc.tensor.matmul(p_z[rg:rg + 16, :], W("giz")[rg:rg + 2, :],
                             t_sur[rg:rg + 2, :], start=True, stop=True,
                             tile_position=(rg, rg))
            nc.tensor.matmul(p_n[rg:rg + 16, :], W("gin")[rg:rg + 2, :],
                             t_sur[rg:rg + 2, :], start=True, stop=True,
                             tile_position=(rg, rg))

        # r = sigmoid(gi_r + br); rp = r*bhh_n
        t_er = tl.tile([128, C], f32, name="t_er", tag="sc", bufs=4)
        nc.scalar.activation(t_er[:], p_r[:], AF.Exp, bias=Bc("rn"), scale=-1.0)
        nc.vector.tensor_scalar(t_er[:], t_er[:], 1.0, None, ALU.add)
        t_r = tl.tile([128, C], f32, name="t_r")
        nc.vector.reciprocal_approx_fast(t_r[:], t_er[:])
        t_rp = tl.tile([128, C], f32, name="t_rp")
        nc.vector.tensor_scalar(t_rp[:], t_r[:], Bc("bhhn"), None, ALU.mult)

        # zc = 1-z = recip(1+exp(+x+bz))
        t_ez = tl.tile([128, C], f32, name="t_ez", tag="sc", bufs=4)
        nc.scalar.activation(t_ez[:], p_z[:], AF.Exp, bias=Bc("z"))
        nc.vector.tensor_scalar(t_ez[:], t_ez[:], 1.0, None, ALU.add)
        t_zc = tl.tile([128, C], f32, name="t_zc")
        nc.vector.reciprocal_approx_fast(t_zc[:], t_ez[:])

        # n = tanh(gi_n + bn + rp) = 1 - 2*recip(1+exp(2t+2bn))
        t_t = tl.tile([128, C], f32, name="t_t")
        nc.vector.tensor_add(t_t[:], p_n[:], t_rp[:])
        t_en = tl.tile([128, C], f32, name="t_en", tag="sc", bufs=4)
        nc.scalar.activation(t_en[:], t_t[:], AF.Exp, bias=Bc("n2"), scale=2.0)
        nc.vector.tensor_scalar(t_en[:], t_en[:], 1.0, None, ALU.add)
        t_w = tl.tile([128, C], f32, name="t_w")
        nc.vector.reciprocal_approx_fast(t_w[:], t_en[:])
        # h = zc*n = zc - 2*zc*w
        t_zw = tl.tile([128, C], f32, name="t_zw")
        nc.vector.tensor_mul(t_zw[:], t_zc[:], t_w[:])
        nc.vector.tensor_scalar(t_zw[:], t_zw[:], -2.0, None, ALU.mult)
        t_h = tl.tile([128, C], f32, name="t_h")
        nc.vector.tensor_add(t_h[:], t_zc[:], t_zw[:])
        t_h2 = tl.tile([128, C], bf16, name="t_h2")
        nc.scalar.activation(t_h2[:], t_h[:], AF.Square)

        p_s = psp.tile([128, C], f32, name="p_s", tag="pp")
        for g in range(4):
            rg = 32 * g
            nc.tensor.matmul(p_s[rg:rg + 1, :], W("ones16")[rg:rg + 16, :],
                             t_h2[rg:rg + 16, :], start=True, stop=True,
                             tile_position=(rg, rg))
        t_ls = tl.tile([128, C], f32, name="t_ls", tag="sc", bufs=4)
        nc.scalar.activation(t_ls[:], p_s[:], AF.Ln)
        t_s = tl.tile([128, C], f32, name="t_s")
        nc.scalar.activation(t_s[:], t_ls[:], AF.Exp, scale=0.5)

        # ignition = sigmoid(alpha*(S - theta))
        t_d = tl.tile([128, C], f32, name="t_d")
        nc.vector.tensor_sub(t_d[:], t_s[:], t_th[:])
        t_ei = tl.tile([128, C], f32, name="t_ei", tag="sc", bufs=4)
        nc.scalar.activation(t_ei[:], t_d[:], AF.Exp, scale=-alpha)
        nc.vector.tensor_scalar(t_ei[:], t_ei[:], 1.0, None, ALU.add)
        t_ign = tl.tile([128, C], f32, name="t_ign")
        nc.vector.reciprocal_approx_fast(t_ign[:], t_ei[:])
        ignb = tl.tile([128, C], bf16, name="ignb")
        nc.vector.tensor_copy(ignb[:], t_ign[:])

        # ---- heads ----
        gat = act.tile([64, F], bf16, name="gat")
        som = act.tile([64, F], f32, name="som")
        so1h = act.tile([64, F], bf16, name="so1h")
        pv = act.tile([64, F], bf16, name="pv")
        expp = tl.tile([128, C], bf16, name="expp")
        pol = tl.tile([128, C], f32, name="pol")
        valp = tl.tile([128, C], f32, name="valp")
        mod = act.tile([64, F], f32, name="mod")
        pin = act.tile([64, F], bf16, name="pin")

        p_den = psp.tile([128, C], f32, name="p_den", tag="pp")

        for g in range(4):
            cs = slice(g * C, (g + 1) * C)
            rg = 32 * g
            pg = psw.tile([64, C], f32, name="pg", tag="pw")
            nc.tensor.matmul(pg[:], W("ones64")[rg:rg + 1, :],
                             ignb[rg:rg + 1, :], start=True, stop=True,
                             tile_position=(rg, 0))
            nc.vector.tensor_mul(gat[:, cs], pg[:], ws[:, cs])

            p7 = psw.tile([96, C], f32, name="p7", tag="pw")
            nc.tensor.matmul(p7[:], W("so1va1"), gat[:, cs], start=True, stop=True)
            nc.vector.tensor_scalar(so1h[:, cs], p7[0:64, :],
                                    Bc("so1va1", slice(0, 64)), 0.0, ALU.add, ALU.max)
            nc.vector.tensor_scalar(pv[32:64, cs], p7[64:96, :],
                                    Bc("so1va1", slice(64, 96)), 0.0, ALU.add, ALU.max)

            p8 = psw.tile([64, C], f32, name="p8", tag="pw")
            nc.tensor.matmul(p8[:], W("so2"), so1h[:, cs], start=True, stop=True)
            nc.scalar.activation(som[:, cs], p8[:], AF.Identity, bias=Bc("so2", slice(0, 64)))
            # sigmoid(somatic): exp(-p8 - b) -> +1 -> recip
            nc.scalar.activation(mod[:, cs], p8[:], AF.Exp,
                                 bias=Bc("so2n", slice(0, 64)), scale=-1.0)
            nc.vector.tensor_scalar(mod[:, cs], mod[:, cs], 1.0, None, ALU.add)
            nc.vector.reciprocal_approx_fast(mod[:, cs], mod[:, cs])
            # pin = gat * (1 + 0.3*sig)
            nc.vector.tensor_scalar(mod[:, cs], mod[:, cs], 0.3, 1.0, ALU.mult, ALU.add)
            nc.vector.tensor_mul(pin[:, cs], gat[:, cs], mod[:, cs])

            p9 = psw.tile([32, C], f32, name="p9", tag="pw")
            nc.tensor.matmul(p9[:], W("po1"), pin[:, cs], start=True, stop=True)
            nc.vector.tensor_scalar(pv[0:32, cs], p9[:], Bc("po1", slice(0, 32)),
                                    0.0, ALU.add, ALU.max)

            p10 = psw.tile([33, C], f32, name="p10", tag="pw")
            nc.tensor.matmul(p10[:], W("po2va2"), pv[:, cs], start=True, stop=True)
            nc.scalar.activation(expp[rg:rg + 16, :], p10[0:16, :], AF.Exp,
                                 bias=Bc("po2", slice(0, 16)))
            nc.vector.tensor_scalar(valp[rg:rg + 1, :], p10[32:33, :],
                                    Bc("va2", slice(32, 33)), None, ALU.add)

            nc.tensor.matmul(p_den[rg:rg + 1, :], W("ones16")[rg:rg + 16, :],
                             expp[rg:rg + 16, :], start=True, stop=True,
                             tile_position=(rg, rg))

        t_rd = tl.tile([128, C], f32, name="t_rd")
        nc.vector.reciprocal_approx_fast(t_rd[:], p_den[:])
        t_rdb = tl.tile([128, C], bf16, name="t_rdb")
        nc.vector.tensor_copy(t_rdb[:], t_rd[:])
        for g in range(4):
            rg = 32 * g
            pdb = psw.tile([16, C], f32, name="pdb", tag="pw")
            nc.tensor.matmul(pdb[:], W("ones64")[rg:rg + 1, 0:16],
                             t_rdb[rg:rg + 1, :], start=True, stop=True,
                             tile_position=(rg, 0))
            nc.vector.tensor_mul(pol[rg:rg + 16, :], pdb[:], expp[rg:rg + 16, :])

        # ---- output DMAs ----
        nc.sync.dma_start(o_som[:, col0:col0 + F], som[:])
        nc.gpsimd.dma_start(o_gat[:, col0:col0 + F], gat[:])
        for g in range(4):
            cs = slice(col0 + g * C, col0 + (g + 1) * C)
            rg = 32 * g
            nc.sync.dma_start(o_pol[:, cs], pol[rg:rg + 16, :])
            nc.sync.dma_start(o_val[:, cs], valp[rg:rg + 1, :])
            nc.sync.dma_start(o_ign[:, cs], t_ign[rg:rg + 1, :])
            nc.sync.dma_start(o_s[:, cs], t_s[rg:rg + 1, :])
            nc.sync.dma_start(o_th[:, cs], t_th[rg:rg + 1, :])
            nc.sync.dma_start(o_pi[:, cs], t_pi[rg:rg + 2, :])

    ctx.close()


_CACHED = {}


def _get_compiled(b_loc, wR_cols, wB_cols, bias_cols, rs, bs, bbs, alpha):
    key = (b_loc, wR_cols, wB_cols, bias_cols)
    if key in _CACHED:
        return _CACHED[key]
    nc = bacc.Bacc()
    ext, nit = build(nc, b_loc, wR_cols, wB_cols, bias_cols, alpha)
    with tile.TileContext(nc) as tc:
        emit(nc, tc, ext, nit, rs, bs, bbs, alpha, wR_cols, wB_cols, bias_cols)
    nc.compile()
    _CACHED[key] = nc
    return nc


def kernel(extero_input, intero_input, context, params, _b_loc=None, _trace=False):
    extero_input = np.asarray(extero_input, np.float32)
    intero_input = np.asarray(intero_input, np.float32)
    context = np.asarray(context, np.float32)
    b = extero_input.shape[0]
    b_loc = _b_loc or b // N_CORES

    wR, rs, wB, bs, bias, bbs, alpha = prep_weights(params)
    nc = _get_compiled(b_loc, wR.shape[1], wB.shape[1], bias.shape[1],
                       rs, bs, bbs, alpha)

    in_maps = []
    for cid in range(N_CORES):
        sl = slice(cid * b_loc, (cid + 1) * b_loc)
        xe = np.ascontiguousarray(extero_input[sl].T)
        xic = np.empty((40, b_loc), np.float32)
        xic[0:32] = intero_input[sl].T
        xic[32:40] = context[sl].T
        in_maps.append({"xe": xe, "xic": xic, "wR": wR, "wB": wB, "bb": bias})

    res = run_bass_kernel_spmd(nc, in_maps, core_ids=list(range(N_CORES)),
                               trace=_trace)
    outs = res.results

    def gather(name):
        return np.concatenate([outs[c][name] for c in range(N_CORES)], axis=1)

    pol = np.ascontiguousarray(gather("pol").T)
    val = np.ascontiguousarray(gather("val").T)
    ign = np.ascontiguousarray(gather("ign").T)
    st = np.ascontiguousarray(gather("st").T)
    th = np.ascontiguousarray(gather("th").T)
    pi = gather("pi2")
    pi_e = np.ascontiguousarray(pi[0:1].T)
    pi_i = np.ascontiguousarray(pi[1:2].T)
    som = np.ascontiguousarray(gather("som").T)
    gat = np.ascontiguousarray(gather("gat").T)
    if _trace:
        kernel._last_exec_time_ns = res.exec_time_ns
    return pol, val, ign, st, th, pi_e, pi_i, som, gat


# revision 11
# speedup vs baseline: 1.3066x; 1.3066x over previous
"""APGI network Bass kernel for 8 TRN2 NeuronCores (pure data parallel).

Layout: feature-major (batch on the free/column axis). Host pre-transposes
inputs per core; device computes all 9 outputs feature-major; host
transposes back.

Self-contained: hardcodes shapes from the problem spec (B=524288, E=128,
I=32, C=8, A=16), 8 cores.
"""
import os
import sys
import numpy as np

sys.path.insert(0, "/opt/trn_rl_repo")

import concourse.bass as bass
import concourse.bacc as bacc
import concourse.tile as tile
from concourse import mybir
from concourse.bass_utils import run_bass_kernel_spmd

# Pin the ACT table to the single set covering every func we use
# (exp, ln, relu, identity, square) so bacc inserts ONE table load
# instead of ping-ponging between exp_and_others and natural_log.
from concourse import hw_specs as _hw_specs
_orig_gat = _hw_specs.get_activation_tables

def _single_set_tables(arch):
    d = _orig_gat(arch)
    return {k: (v if k == "natural_log_exp_and_others" else set())
            for k, v in d.items()}

bacc.get_activation_tables = _single_set_tables

AF = mybir.ActivationFunctionType
ALU = mybir.AluOpType
f32 = mybir.dt.float32
f32r = mybir.dt.float32r
bf16 = mybir.dt.bfloat16

B = 524288
N_CORES = 8
B_LOC = B // N_CORES

# Column tiling: F batch columns per iteration, split into 4 chunks of C.
F = 2048
C = F // 4


# ---------------------------------------------------------------------------
# Host-side weight packing
# ---------------------------------------------------------------------------
def _rep4(vals, offs, width=1):
    """Replicate a per-row pattern into all four 32-row blocks.

    vals: [n, width]; placed at rows 32g+offs .. 32g+offs+n for g in 0..3.
    """
    out = np.zeros((128, width), np.float32)
    v = np.asarray(vals, np.float32).reshape(-1, width)
    n = v.shape[0]
    for g in range(4):
        out[32 * g + offs:32 * g + offs + n, :] = v
    return out


def prep_weights(p):
    """Pack params dict into wR (f32r lhsT blob), wB (bf16 lhsT blob),
    bias blob, plus slice metadata. All np.float32 host-side."""
    P = {k: np.asarray(v, np.float32) for k, v in p.items()}
    alpha = float(abs(np.float32(P["alpha"])))
    beta = float(abs(np.float32(P["beta"])))

    # --- f32r blob: e1 and i1th1 (input layers) ---
    wR = np.zeros((128, 128 + 80), np.float32)
    wR[0:128, 0:128] = P["ew1"].T                     # e1 lhsT [128,128]
    wR[0:32, 128:192] = P["iw1"].T                    # i1 [32,64]
    wR[32:40, 192:208] = P["th_w1"].T                 # th1 [8,16]
    rs = {"e1": (slice(0, 128), slice(0, 128)),
          "i1th1": (slice(0, 40), slice(128, 208))}

    # --- bf16 blob ---
    cols = []
    bs = {}

    def add(name, rows, mat):
        off = sum(c.shape[1] for c in cols)
        m = np.zeros((128, mat.shape[1]), np.float32)
        m[rows, :] = mat
        cols.append(m)
        bs[name] = (rows, slice(off, off + mat.shape[1]))

    add("e2", slice(0, 128), P["ew2"].T)              # [128,64]
    add("i2", slice(0, 64), P["iw2"].T)               # [64,32]
    m = np.zeros((96, 48), np.float32)
    m[0:64, 0:32] = P["ew3"].T
    m[64:96, 32:48] = P["iw3"].T
    add("e3i3", slice(0, 96), m)
    m = np.zeros((48, 88), np.float32)
    m[:, 0:64] = P["ws_w"].T
    m[0:32, 64:80] = P["pe_w1"].T
    m[32:48, 80:88] = P["pi_w1"].T
    add("wspepi", slice(0, 48), m)
    m = np.zeros((64, 96), np.float32)
    m[:, 0:64] = P["so_w1"].T
    m[:, 64:96] = P["va_w1"].T
    add("so1va1", slice(0, 64), m)
    add("so2", slice(0, 64), P["so_w2"].T)
    add("po1", slice(0, 64), P["po_w1"].T)
    m = np.zeros((64, 33), np.float32)
    m[0:32, 0:16] = P["po_w2"].T
    m[32:64, 32:33] = P["va_w2"].T
    add("po2va2", slice(0, 64), m)
    # packed lhsTs
    m = np.zeros((48, 2), np.float32)
    m[0:32, 0] = 1.0
    m[32:48, 1] = 1.0
    add("epsones", slice(0, 48), m)                   # sum-sq reduce
    m = np.zeros((24, 2), np.float32)
    m[0:16, 0:1] = P["pe_w2"].T
    m[16:24, 1:2] = P["pi_w2"].T
    add("pepi2", slice(0, 128), _repmat(m, 24))
    add("th2", slice(0, 128), _repmat(P["th_w2"].T, 16))
    wih = P["gru_wih"]                                # [48,2]
    add("gir", slice(0, 128), _repmat(wih[0:16].T, 2))
    add("giz", slice(0, 128), _repmat(wih[16:32].T, 2))
    add("gin", slice(0, 128), _repmat(wih[32:48].T, 2))
    add("ones16", slice(0, 128), _repmat(np.ones((16, 1), np.float32), 16))
    add("ones64", slice(0, 128), np.ones((128, 64), np.float32))
    wB = np.concatenate(cols, axis=1)

    # --- bias blob (f32, one [128,1] column per entry) ---
    bcols = []
    bbs = {}

    def addb(name, col):
        bbs[name] = (sum(c.shape[1] for c in bcols),)
        bcols.append(np.asarray(col, np.float32).reshape(128, 1))

    def at(rows, vals):
        c = np.zeros((128, 1), np.float32)
        c[rows, 0] = vals
        return c

    addb("e1", at(slice(0, 128), P["eb1"]))
    c2 = np.zeros((128, 1), np.float32)
    c2[0:64, 0] = P["ib1"]
    c2[64:80, 0] = P["th_b1"]
    addb("i1th1", c2)
    addb("e2", at(slice(0, 64), P["eb2"]))
    addb("i2", at(slice(0, 32), P["ib2"]))
    c2 = np.zeros((128, 1), np.float32)
    c2[0:32, 0] = P["eb3"]
    c2[32:48, 0] = P["ib3"]
    addb("c48", c2)
    c2 = np.zeros((128, 1), np.float32)
    c2[0:64, 0] = P["ws_b"]
    c2[64:80, 0] = P["pe_b1"]
    c2[80:88, 0] = P["pi_b1"]
    addb("wspepi", c2)
    c2 = np.zeros((128, 1), np.float32)
    c2[0:64, 0] = P["so_b1"]
    c2[64:96, 0] = P["va_b1"]
    addb("so1va1", c2)
    addb("so2", at(slice(0, 64), P["so_b2"]))
    addb("so2n", at(slice(0, 64), -P["so_b2"]))
    addb("po1", at(slice(0, 32), P["po_b1"]))
    addb("po2", at(slice(0, 16), P["po_b2"]))
    addb("va2", at(slice(32, 33), P["va_b2"]))
    addb("eps", _rep4([[0.0], [np.log(beta)]], 0))
    addb("pi", _rep4([[P["pe_b2"][0]], [P["pi_b2"][0]]], 0))
    addb("thn", _rep4([[-P["th_b2"][0]]], 0))
    bih, bhh = P["gru_bih"], P["gru_bhh"]
    addb("rn", _rep4(-(bih[0:16] + bhh[0:16]).reshape(-1, 1), 0))
    addb("z", _rep4((bih[16:32] + bhh[16:32]).reshape(-1, 1), 0))
    addb("n2", _rep4((2.0 * bih[32:48]).reshape(-1, 1), 0))
    addb("bhhn", _rep4(bhh[32:48].reshape(-1, 1), 0))
    bias = np.concatenate(bcols, axis=1)

    return wR, rs, wB, bs, bias, bbs, alpha


def _repmat(m, nrows):
    """Replicate [nrows, w] matrix into rows 32g..32g+nrows for g=0..3."""
    out = np.zeros((128, m.shape[1]), np.float32)
    for g in range(4):
        out[32 * g:32 * g + nrows, :] = m
    return out


# ---------------------------------------------------------------------------
# Device kernel builder
# ---------------------------------------------------------------------------
def build(nc, b_loc, wR_cols, wB_cols, bias_cols, alpha):
    nit = b_loc // F

    xe_ext = nc.declare_dram_parameter("xe", [128, b_loc], f32, isOutput=False)
    xic_ext = nc.declare_dram_parameter("xic", [40, b_loc], f32, isOutput=False)
    wR_ext = nc.declare_dram_parameter("wR", [128, wR_cols], f32, isOutput=False)
    wB_ext = nc.declare_dram_parameter("wB", [128, wB_cols], f32, isOutput=False)
    bb_ext = nc.declare_dram_parameter("bb", [128, bias_cols], f32, isOutput=False)

    o_pol = nc.declare_dram_parameter("pol", [16, b_loc], bf16, isOutput=True)
    o_val = nc.declare_dram_parameter("val", [1, b_loc], f32, isOutput=True)
    o_ign = nc.declare_dram_parameter("ign", [1, b_loc], f32, isOutput=True)
    o_s = nc.declare_dram_parameter("st", [1, b_loc], f32, isOutput=True)
    o_th = nc.declare_dram_parameter("th", [1, b_loc], f32, isOutput=True)
    o_pi = nc.declare_dram_parameter("pi2", [2, b_loc], f32, isOutput=True)
    o_som = nc.declare_dram_parameter("som", [64, b_loc], bf16, isOutput=True)
    o_gat = nc.declare_dram_parameter("gat", [64, b_loc], bf16, isOutput=True)

    return (xe_ext, xic_ext, wR_ext, wB_ext, bb_ext,
            o_pol, o_val, o_ign, o_s, o_th, o_pi, o_som, o_gat), nit


def emit(nc, tc, ext, nit, rs, bs, bbs, alpha, wR_cols, wB_cols, bias_cols):
    (xe_ext, xic_ext, wR_ext, wB_ext, bb_ext,
     o_pol, o_val, o_ign, o_s, o_th, o_pi, o_som, o_gat) = ext

    import contextlib
    ctx = contextlib.ExitStack()
    wpool = ctx.enter_context(tc.tile_pool(name="w", bufs=1))
    inp = ctx.enter_context(tc.tile_pool(name="inp", bufs=2))
    act = ctx.enter_context(tc.tile_pool(name="act", bufs=1))
    tl = ctx.enter_context(tc.tile_pool(name="tl", bufs=1))
    psw = ctx.enter_context(tc.tile_pool(name="psw", bufs=3, space="PSUM"))
    psp = ctx.enter_context(tc.tile_pool(name="psp", bufs=3, space="PSUM"))

    # --- weights / biases, loaded once ---
    wR = wpool.tile([128, wR_cols], f32r, name="wR")
    nc.gpsimd.dma_start(wR[:], wR_ext[:])
    wBt = wpool.tile([128, wB_cols], bf16, name="wBt")
    nc.gpsimd.dma_start(wBt[:], wB_ext[:])
    bb = wpool.tile([128, bias_cols], f32, name="bb")
    nc.sync.dma_start(bb[:], bb_ext[:])

    def W(name):
        r, c = bs[name]
        return wBt[r, c]

    def Bc(name, rows=slice(0, 128)):
        return bb[rows, bbs[name][0]:bbs[name][0] + 1]

    for it in range(nit):
        col0 = it * F
        xe = inp.tile([128, F], f32r, name="xe")
        nc.gpsimd.dma_start(xe[:], xe_ext[:, col0:col0 + F])
        xic = inp.tile([40, F], f32r, name="xic")
        nc.gpsimd.dma_start(xic[:], xic_ext[:, col0:col0 + F])

        h1e = act.tile([128, F], bf16, name="h1e", bufs=2)
        h1i = act.tile([64, F], bf16, name="h1i")
        th1h = tl.tile([128, C], bf16, name="th1h", bufs=2)
        h2 = act.tile([96, F], bf16, name="h2")
        c48 = act.tile([48, F], bf16, name="c48", bufs=2)
        c48sq = act.tile([48, F], bf16, name="c48sq")
        ws = act.tile([64, F], f32, name="ws", bufs=2)
        pepih = tl.tile([128, C], bf16, name="pepih", bufs=2)

        p_eps = psp.tile([128, C], f32, name="p_eps", tag="pp")
        p_pi = psp.tile([128, C], f32, name="p_pi", tag="pp")
        p_th = psp.tile([128, C], f32, name="p_th", tag="pp")

        for g in range(4):
            cs = slice(g * C, (g + 1) * C)
            rg = 32 * g
            # ---- encoders ----
            p1 = psw.tile([128, C], f32, name="p1", tag="pw")
            nc.tensor.matmul(p1[:], wR[rs["e1"][0], rs["e1"][1]], xe[:, cs],
                             start=True, stop=True)
            nc.scalar.activation(h1e[:, cs], p1[:], AF.Relu, bias=Bc("e1"))

            p2 = psw.tile([80, C], f32, name="p2", tag="pw")
            nc.tensor.matmul(p2[:], wR[rs["i1th1"][0], rs["i1th1"][1]],
                             xic[:, cs], start=True, stop=True)
            nc.scalar.activation(h1i[:, cs], p2[0:64, :], AF.Relu,
                                 bias=Bc("i1th1", slice(0, 64)))
            nc.scalar.activation(th1h[rg:rg + 16, :], p2[64:80, :], AF.Relu,
                                 bias=Bc("i1th1", slice(64, 80)))

            p3 = psw.tile([64, C], f32, name="p3", tag="pw")
            nc.tensor.matmul(p3[:], W("e2"), h1e[:, cs], start=True, stop=True)
            nc.scalar.activation(h2[0:64, cs], p3[:], AF.Relu,
                                 bias=Bc("e2", slice(0, 64)))

            p4 = psw.tile([32, C], f32, name="p4", tag="pw")
            nc.tensor.matmul(p4[:], W("i2"), h1i[:, cs], start=True, stop=True)
            nc.scalar.activation(h2[64:96, cs], p4[:], AF.Relu,
                                 bias=Bc("i2", slice(0, 32)))

            p5 = psw.tile([48, C], f32, name="p5", tag="pw")
            nc.tensor.matmul(p5[:], W("e3i3"), h2[:, cs], start=True, stop=True)
            nc.scalar.activation(c48[:, cs], p5[:], AF.Identity, bias=Bc("c48", slice(0, 48)))
            nc.scalar.activation(c48sq[:, cs], p5[:], AF.Square, bias=Bc("c48", slice(0, 48)))

            p6 = psw.tile([88, C], f32, name="p6", tag="pw")
            nc.tensor.matmul(p6[:], W("wspepi"), c48[:, cs], start=True, stop=True)
            nc.vector.tensor_scalar(ws[:, cs], p6[0:64, :],
                                    Bc("wspepi", slice(0, 64)), None, ALU.add)
            nc.scalar.activation(pepih[rg:rg + 24, :], p6[64:88, :], AF.Relu,
                                 bias=Bc("wspepi", slice(64, 88)))

        # ---- packed reductions (bf16, tile_position), grouped to limit
        # PE mode switches ----
        for g in range(4):
            cs = slice(g * C, (g + 1) * C)
            rg = 32 * g
            nc.tensor.matmul(p_eps[rg:rg + 2, :], W("epsones")[0:48, :],
                             c48sq[:, cs], start=True, stop=True,
                             tile_position=(0, rg))
            nc.tensor.matmul(p_pi[rg:rg + 2, :], W("pepi2")[rg:rg + 24, :],
                             pepih[rg:rg + 24, :], start=True, stop=True,
                             tile_position=(rg, rg))
            nc.tensor.matmul(p_th[rg:rg + 1, :], W("th2")[rg:rg + 16, :],
                             th1h[rg:rg + 16, :], start=True, stop=True,
                             tile_position=(rg, rg))

        # ---- narrow tail, packed x4 [128, C] ----
        t_ln = tl.tile([128, C], f32, name="t_ln", tag="sc", bufs=4)
        nc.scalar.activation(t_ln[:], p_eps[:], AF.Ln)
        t_eps = tl.tile([128, C], f32, name="t_eps")
        nc.scalar.activation(t_eps[:], t_ln[:], AF.Exp, bias=Bc("eps"), scale=0.5)

        t_pe = tl.tile([128, C], f32, name="t_pe", tag="sc", bufs=4)
        nc.scalar.activation(t_pe[:], p_pi[:], AF.Exp, bias=Bc("pi"))
        t_pi = tl.tile([128, C], f32, name="t_pi", bufs=2)
        nc.scalar.activation(t_pi[:], t_pe[:], AF.Ln, bias=1.0)

        t_sur = tl.tile([128, C], bf16, name="t_sur", bufs=2)
        nc.vector.tensor_mul(t_sur[:], t_eps[:], t_pi[:])

        # theta = sigmoid(th2+b) = recip(1+exp(-x-b))
        t_te = tl.tile([128, C], f32, name="t_te", tag="sc", bufs=4)
        nc.scalar.activation(t_te[:], p_th[:], AF.Exp, bias=Bc("thn"), scale=-1.0)
        nc.vector.tensor_scalar(t_te[:], t_te[:], 1.0, None, ALU.add)
        t_th = tl.tile([128, C], f32, name="t_th", bufs=2)
        nc.vector.reciprocal_approx_fast(t_th[:], t_te[:])

        # GRU gates
        p_r = psp.tile([128, C], f32, name="p_r", tag="pp")
        p_z = psp.tile([128, C], f32, name="p_z", tag="pp")
        p_n = psp.tile([128, C], f32, name="p_n", tag="pp")
        for g in range(4):
            rg = 32 * g
            nc.tensor.matmul(p_r[rg:rg + 16, :], W("gir")[rg:rg + 2, :],
                             t_sur[rg:rg + 2, :], start=True, stop=True,
                             tile_position=(rg, rg))
            nc.tensor.matmul(p_z[rg:rg + 16, :], W("giz")[rg:rg + 2, :],
                             t_sur[rg:rg + 2, :], start=True, stop=True,
                             tile_position=(rg, rg))
            nc.tensor.matmul(p_n[rg:rg + 16, :], W("gin")[rg:rg + 2, :],
                             t_sur[rg:rg + 2, :], start=True, stop=True,
                             tile_position=(rg, rg))

        # r = sigmoid(gi_r + br); rp = r*bhh_n
        t_er = tl.tile([128, C], f32, name="t_er", tag="sc", bufs=4)
        nc.scalar.activation(t_er[:], p_r[:], AF.Exp, bias=Bc("rn"), scale=-1.0)
        nc.vector.tensor_scalar(t_er[:], t_er[:], 1.0, None, ALU.add)
        t_r = tl.tile([128, C], f32, name="t_r")
        nc.vector.reciprocal_approx_fast(t_r[:], t_er[:])


        # zc = 1-z = recip(1+exp(+x+bz))
        t_ez = tl.tile([128, C], f32, name="t_ez", tag="sc", bufs=4)
        nc.scalar.activation(t_ez[:], p_z[:], AF.Exp, bias=Bc("z"))
        nc.vector.tensor_scalar(t_ez[:], t_ez[:], 1.0, None, ALU.add)
        t_zc = tl.tile([128, C], f32, name="t_zc")
        nc.vector.reciprocal_approx_fast(t_zc[:], t_ez[:])

        # n = tanh(gi_n + bn + r*bhh_n) = 1 - 2*recip(1+exp(2t+2bn))
        t_t = tl.tile([128, C], f32, name="t_t")
        nc.vector.scalar_tensor_tensor(t_t[:], t_r[:], Bc("bhhn"), p_n[:],
                                       ALU.mult, ALU.add)
        t_en = tl.tile([128, C], f32, name="t_en", tag="sc", bufs=4)
        nc.scalar.activation(t_en[:], t_t[:], AF.Exp, bias=Bc("n2"), scale=2.0)
        nc.vector.tensor_scalar(t_en[:], t_en[:], 1.0, None, ALU.add)
        t_w = tl.tile([128, C], f32, name="t_w")
        nc.vector.reciprocal_approx_fast(t_w[:], t_en[:])
        # h = zc*n = zc - 2*zc*w
        t_zw = tl.tile([128, C], f32, name="t_zw")
        nc.vector.scalar_tensor_tensor(t_zw[:], t_w[:], -2.0, t_zc[:],
                                       ALU.mult, ALU.mult)
        t_h = tl.tile([128, C], f32, name="t_h")
        nc.vector.tensor_add(t_h[:], t_zc[:], t_zw[:])
        t_h2 = tl.tile([128, C], bf16, name="t_h2")
        nc.scalar.activation(t_h2[:], t_h[:], AF.Square)

        p_s = psp.tile([128, C], f32, name="p_s", tag="pp")
        for g in range(4):
            rg = 32 * g
            nc.tensor.matmul(p_s[rg:rg + 1, :], W("ones16")[rg:rg + 16, :],
                             t_h2[rg:rg + 16, :], start=True, stop=True,
                             tile_position=(rg, rg))
        t_ls = tl.tile([128, C], f32, name="t_ls", tag="sc", bufs=4)
        nc.scalar.activation(t_ls[:], p_s[:], AF.Ln)
        t_s = tl.tile([128, C], f32, name="t_s", bufs=2)
        nc.scalar.activation(t_s[:], t_ls[:], AF.Exp, scale=0.5)

        # ignition = sigmoid(alpha*(S - theta))
        t_d = tl.tile([128, C], f32, name="t_d")
        nc.vector.tensor_sub(t_d[:], t_s[:], t_th[:])
        t_ei = tl.tile([128, C], f32, name="t_ei", tag="sc", bufs=4)
        nc.scalar.activation(t_ei[:], t_d[:], AF.Exp, scale=-alpha)
        nc.vector.tensor_scalar(t_ei[:], t_ei[:], 1.0, None, ALU.add)
        t_ign = tl.tile([128, C], f32, name="t_ign", bufs=2)
        nc.vector.reciprocal_approx_fast(t_ign[:], t_ei[:])
        ignb = tl.tile([128, C], bf16, name="ignb")
        nc.vector.tensor_copy(ignb[:], t_ign[:])

        # ---- heads ----
        gat = act.tile([64, F], bf16, name="gat", bufs=2)
        som = act.tile([64, F], bf16, name="som", bufs=2)
        so1h = act.tile([64, F], bf16, name="so1h")
        pv = act.tile([64, F], bf16, name="pv")
        expp = tl.tile([128, C], bf16, name="expp", bufs=2)
        pol = tl.tile([128, C], bf16, name="pol", bufs=2)
        valp = tl.tile([128, C], f32, name="valp", bufs=2)
        mod = act.tile([64, F], f32, name="mod")
        pin = act.tile([64, F], bf16, name="pin")

        p_den = psp.tile([128, C], f32, name="p_den", tag="pp")

        for g in range(4):
            cs = slice(g * C, (g + 1) * C)
            rg = 32 * g
            pg = psw.tile([64, C], f32, name="pg", tag="pg", bufs=2)
            nc.tensor.matmul(pg[:], W("ones64")[rg:rg + 1, :],
                             ignb[rg:rg + 1, :], start=True, stop=True,
                             tile_position=(rg, 0))
            nc.vector.tensor_mul(gat[:, cs], pg[:], ws[:, cs])

            p7 = psw.tile([96, C], f32, name="p7", tag="pw")
            nc.tensor.matmul(p7[:], W("so1va1"), gat[:, cs], start=True, stop=True)
            nc.vector.tensor_scalar(so1h[:, cs], p7[0:64, :],
                                    Bc("so1va1", slice(0, 64)), 0.0, ALU.add, ALU.max)
            nc.vector.tensor_scalar(pv[32:64, cs], p7[64:96, :],
                                    Bc("so1va1", slice(64, 96)), 0.0, ALU.add, ALU.max)

            p8 = psw.tile([64, C], f32, name="p8", tag="pw")
            nc.tensor.matmul(p8[:], W("so2"), so1h[:, cs], start=True, stop=True)
            nc.scalar.activation(som[:, cs], p8[:], AF.Identity, bias=Bc("so2", slice(0, 64)))
            # sigmoid(somatic): exp(-p8 - b) -> +1 -> recip
            nc.scalar.activation(mod[:, cs], p8[:], AF.Exp,
                                 bias=Bc("so2n", slice(0, 64)), scale=-1.0)
            nc.vector.tensor_scalar(mod[:, cs], mod[:, cs], 1.0, None, ALU.add)
            nc.vector.reciprocal_approx_fast(mod[:, cs], mod[:, cs])
            # pin = gat * (1 + 0.3*sig)
            nc.vector.tensor_scalar(mod[:, cs], mod[:, cs], 0.3, 1.0, ALU.mult, ALU.add)
            nc.vector.tensor_mul(pin[:, cs], gat[:, cs], mod[:, cs])

            p9 = psw.tile([32, C], f32, name="p9", tag="pw")
            nc.tensor.matmul(p9[:], W("po1"), pin[:, cs], start=True, stop=True)
            nc.vector.tensor_scalar(pv[0:32, cs], p9[:], Bc("po1", slice(0, 32)),
                                    0.0, ALU.add, ALU.max)

            p10 = psw.tile([33, C], f32, name="p10", tag="pw")
            nc.tensor.matmul(p10[:], W("po2va2"), pv[:, cs], start=True, stop=True)
            nc.scalar.activation(expp[rg:rg + 16, :], p10[0:16, :], AF.Exp,
                                 bias=Bc("po2", slice(0, 16)))
            nc.vector.tensor_scalar(valp[rg:rg + 1, :], p10[32:33, :],
                                    Bc("va2", slice(32, 33)), None, ALU.add)

            nc.tensor.matmul(p_den[rg:rg + 1, :], W("ones16")[rg:rg + 16, :],
                             expp[rg:rg + 16, :], start=True, stop=True,
                             tile_position=(rg, rg))

        t_rd = tl.tile([128, C], f32, name="t_rd")
        nc.vector.reciprocal_approx_fast(t_rd[:], p_den[:])
        t_rdb = tl.tile([128, C], bf16, name="t_rdb")
        nc.vector.tensor_copy(t_rdb[:], t_rd[:])
        for g in range(4):
            rg = 32 * g
            pdb = psw.tile([16, C], f32, name="pdb", tag="pw")
            nc.tensor.matmul(pdb[:], W("ones64")[rg:rg + 1, 0:16],
                             t_rdb[rg:rg + 1, :], start=True, stop=True,
                             tile_position=(rg, 0))
            nc.vector.tensor_mul(pol[rg:rg + 16, :], pdb[:], expp[rg:rg + 16, :])

        # ---- output DMAs ----
        nc.sync.dma_start(o_som[:, col0:col0 + F], som[:])
        nc.sync.dma_start(o_gat[:, col0:col0 + F], gat[:])
        for g in range(4):
            cs = slice(col0 + g * C, col0 + (g + 1) * C)
            rg = 32 * g
            nc.sync.dma_start(o_pol[:, cs], pol[rg:rg + 16, :])
            nc.scalar.dma_start(o_val[:, cs], valp[rg:rg + 1, :])
            nc.scalar.dma_start(o_ign[:, cs], t_ign[rg:rg + 1, :])
            nc.sync.dma_start(o_s[:, cs], t_s[rg:rg + 1, :])
            nc.scalar.dma_start(o_th[:, cs], t_th[rg:rg + 1, :])
            nc.sync.dma_start(o_pi[:, cs], t_pi[rg:rg + 2, :])

    ctx.close()


_CACHED = {}


def _get_compiled(b_loc, wR_cols, wB_cols, bias_cols, rs, bs, bbs, alpha):
    key = (b_loc, wR_cols, wB_cols, bias_cols)
    if key in _CACHED:
        return _CACHED[key]
    nc = bacc.Bacc()
    ext, nit = build(nc, b_loc, wR_cols, wB_cols, bias_cols, alpha)
    with tile.TileContext(nc) as tc:
        emit(nc, tc, ext, nit, rs, bs, bbs, alpha, wR_cols, wB_cols, bias_cols)
    nc.compile()
    _CACHED[key] = nc
    return nc


def kernel(extero_input, intero_input, context, params, _b_loc=None, _trace=False):
    extero_input = np.asarray(extero_input, np.float32)
    intero_input = np.asarray(intero_input, np.float32)
    context = np.asarray(context, np.float32)
    b = extero_input.shape[0]
    b_loc = _b_loc or b // N_CORES

    wR, rs, wB, bs, bias, bbs, alpha = prep_weights(params)
    nc = _get_compiled(b_loc, wR.shape[1], wB.shape[1], bias.shape[1],
                       rs, bs, bbs, alpha)

    in_maps = []
    for cid in range(N_CORES):
        sl = slice(cid * b_loc, (cid + 1) * b_loc)
        xe = np.ascontiguousarray(extero_input[sl].T)
        xic = np.empty((40, b_loc), np.float32)
        xic[0:32] = intero_input[sl].T
        xic[32:40] = context[sl].T
        in_maps.append({"xe": xe, "xic": xic, "wR": wR, "wB": wB, "bb": bias})

    res = run_bass_kernel_spmd(nc, in_maps, core_ids=list(range(N_CORES)),
                               trace=_trace)
    outs = res.results

    def gather(name):
        return np.concatenate([outs[c][name] for c in range(N_CORES)], axis=1)

    pol = np.ascontiguousarray(gather("pol").T.astype(np.float32))
    val = np.ascontiguousarray(gather("val").T)
    ign = np.ascontiguousarray(gather("ign").T)
    st = np.ascontiguousarray(gather("st").T)
    th = np.ascontiguousarray(gather("th").T)
    pi = gather("pi2")
    pi_e = np.ascontiguousarray(pi[0:1].T)
    pi_i = np.ascontiguousarray(pi[1:2].T)
    som = np.ascontiguousarray(gather("som").T.astype(np.float32))
    gat = np.ascontiguousarray(gather("gat").T.astype(np.float32))
    if _trace:
        kernel._last_exec_time_ns = res.exec_time_ns
        kernel._last_res = res
    return pol, val, ign, st, th, pi_e, pi_i, som, gat


# revision 12
# speedup vs baseline: 1.8064x; 1.3825x over previous
"""APGI network Bass kernel for 8 TRN2 NeuronCores (pure data parallel).

Layout: feature-major (batch on the free/column axis). Host pre-transposes
inputs per core; device computes all 9 outputs feature-major; host
transposes back.

Self-contained: hardcodes shapes from the problem spec (B=524288, E=128,
I=32, C=8, A=16), 8 cores.
"""
import os
import sys
import numpy as np

sys.path.insert(0, "/opt/trn_rl_repo")

import concourse.bass as bass
import concourse.bacc as bacc
import concourse.tile as tile
from concourse import mybir
from concourse.bass_utils import run_bass_kernel_spmd

# Pin the ACT table to the single set covering every func we use
# (exp, ln, relu, identity, square) so bacc inserts ONE table load
# instead of ping-ponging between exp_and_others and natural_log.
from concourse import hw_specs as _hw_specs
_orig_gat = _hw_specs.get_activation_tables

def _single_set_tables(arch):
    d = _orig_gat(arch)
    return {k: (v if k == "natural_log_exp_and_others" else set())
            for k, v in d.items()}

bacc.get_activation_tables = _single_set_tables

AF = mybir.ActivationFunctionType
ALU = mybir.AluOpType
f32 = mybir.dt.float32
f32r = mybir.dt.float32r
bf16 = mybir.dt.bfloat16

B = 524288
N_CORES = 8
B_LOC = B // N_CORES

# Column tiling: F batch columns per iteration, split into 4 chunks of C.
F = 2048
C = F // 4


# ---------------------------------------------------------------------------
# Host-side weight packing
# ---------------------------------------------------------------------------
def _rep4(vals, offs, width=1):
    """Replicate a per-row pattern into all four 32-row blocks.

    vals: [n, width]; placed at rows 32g+offs .. 32g+offs+n for g in 0..3.
    """
    out = np.zeros((128, width), np.float32)
    v = np.asarray(vals, np.float32).reshape(-1, width)
    n = v.shape[0]
    for g in range(4):
        out[32 * g + offs:32 * g + offs + n, :] = v
    return out


def prep_weights(p):
    """Pack params dict into wR (f32r lhsT blob), wB (bf16 lhsT blob),
    bias blob, plus slice metadata. All np.float32 host-side."""
    P = {k: np.asarray(v, np.float32) for k, v in p.items()}
    alpha = float(abs(np.float32(P["alpha"])))
    beta = float(abs(np.float32(P["beta"])))

    # --- f32r blob: e1 and i1th1 (input layers) ---
    wR = np.zeros((128, 128 + 80), np.float32)
    wR[0:128, 0:128] = P["ew1"].T                     # e1 lhsT [128,128]
    wR[0:32, 128:192] = P["iw1"].T                    # i1 [32,64]
    wR[32:40, 192:208] = P["th_w1"].T                 # th1 [8,16]
    rs = {"e1": (slice(0, 128), slice(0, 128)),
          "i1th1": (slice(0, 40), slice(128, 208))}

    # --- bf16 blob ---
    cols = []
    bs = {}

    def add(name, rows, mat):
        off = sum(c.shape[1] for c in cols)
        m = np.zeros((128, mat.shape[1]), np.float32)
        m[rows, :] = mat
        cols.append(m)
        bs[name] = (rows, slice(off, off + mat.shape[1]))

    add("e2", slice(0, 128), P["ew2"].T)              # [128,64]
    add("i2", slice(0, 64), P["iw2"].T)               # [64,32]
    m = np.zeros((96, 48), np.float32)
    m[0:64, 0:32] = P["ew3"].T
    m[64:96, 32:48] = P["iw3"].T
    add("e3i3", slice(0, 96), m)
    m = np.zeros((48, 88), np.float32)
    m[:, 0:64] = P["ws_w"].T
    m[0:32, 64:80] = P["pe_w1"].T
    m[32:48, 80:88] = P["pi_w1"].T
    add("wspepi", slice(0, 48), m)
    m = np.zeros((64, 96), np.float32)
    m[:, 0:64] = P["so_w1"].T
    m[:, 64:96] = P["va_w1"].T
    add("so1va1", slice(0, 64), m)
    add("so2", slice(0, 64), P["so_w2"].T)
    add("po1", slice(0, 64), P["po_w1"].T)
    m = np.zeros((64, 33), np.float32)
    m[0:32, 0:16] = P["po_w2"].T
    m[32:64, 32:33] = P["va_w2"].T
    add("po2va2", slice(0, 64), m)
    # packed lhsTs
    m = np.zeros((48, 2), np.float32)
    m[0:32, 0] = 1.0
    m[32:48, 1] = 1.0
    add("epsones", slice(0, 48), m)                   # sum-sq reduce
    m = np.zeros((24, 2), np.float32)
    m[0:16, 0:1] = P["pe_w2"].T
    m[16:24, 1:2] = P["pi_w2"].T
    add("pepi2", slice(0, 128), _repmat(m, 24))
    add("th2", slice(0, 128), _repmat(P["th_w2"].T, 16))
    wih = P["gru_wih"]                                # [48,2]
    add("gir", slice(0, 128), _repmat(wih[0:16].T, 2))
    add("giz", slice(0, 128), _repmat(wih[16:32].T, 2))
    add("gin", slice(0, 128), _repmat(wih[32:48].T, 2))
    add("ones16", slice(0, 128), _repmat(np.ones((16, 1), np.float32), 16))
    add("ones64", slice(0, 128), np.ones((128, 64), np.float32))
    wB = np.concatenate(cols, axis=1)

    # --- bias blob (f32, one [128,1] column per entry) ---
    bcols = []
    bbs = {}

    def addb(name, col):
        bbs[name] = (sum(c.shape[1] for c in bcols),)
        bcols.append(np.asarray(col, np.float32).reshape(128, 1))

    def at(rows, vals):
        c = np.zeros((128, 1), np.float32)
        c[rows, 0] = vals
        return c

    addb("e1", at(slice(0, 128), P["eb1"]))
    c2 = np.zeros((128, 1), np.float32)
    c2[0:64, 0] = P["ib1"]
    c2[64:80, 0] = P["th_b1"]
    addb("i1th1", c2)
    addb("e2", at(slice(0, 64), P["eb2"]))
    addb("i2", at(slice(0, 32), P["ib2"]))
    c2 = np.zeros((128, 1), np.float32)
    c2[0:32, 0] = P["eb3"]
    c2[32:48, 0] = P["ib3"]
    addb("c48", c2)
    c2 = np.zeros((128, 1), np.float32)
    c2[0:64, 0] = P["ws_b"]
    c2[64:80, 0] = P["pe_b1"]
    c2[80:88, 0] = P["pi_b1"]
    addb("wspepi", c2)
    c2 = np.zeros((128, 1), np.float32)
    c2[0:64, 0] = P["so_b1"]
    c2[64:96, 0] = P["va_b1"]
    addb("so1va1", c2)
    addb("so2", at(slice(0, 64), P["so_b2"]))
    addb("so2n", at(slice(0, 64), -P["so_b2"]))
    addb("po1", at(slice(0, 32), P["po_b1"]))
    addb("po2", at(slice(0, 16), P["po_b2"]))
    addb("va2", at(slice(32, 33), P["va_b2"]))
    addb("eps", _rep4([[0.0], [np.log(beta)]], 0))
    addb("pi", _rep4([[P["pe_b2"][0]], [P["pi_b2"][0]]], 0))
    addb("thn", _rep4([[-P["th_b2"][0]]], 0))
    bih, bhh = P["gru_bih"], P["gru_bhh"]
    addb("rn", _rep4(-(bih[0:16] + bhh[0:16]).reshape(-1, 1), 0))
    addb("z", _rep4((bih[16:32] + bhh[16:32]).reshape(-1, 1), 0))
    addb("n2", _rep4((2.0 * bih[32:48]).reshape(-1, 1), 0))
    addb("bhhn", _rep4(bhh[32:48].reshape(-1, 1), 0))
    bias = np.concatenate(bcols, axis=1)

    return wR, rs, wB, bs, bias, bbs, alpha


def _repmat(m, nrows):
    """Replicate [nrows, w] matrix into rows 32g..32g+nrows for g=0..3."""
    out = np.zeros((128, m.shape[1]), np.float32)
    for g in range(4):
        out[32 * g:32 * g + nrows, :] = m
    return out


# ---------------------------------------------------------------------------
# Device kernel builder
# ---------------------------------------------------------------------------
def build(nc, b_loc, wR_cols, wB_cols, bias_cols, alpha):
    nit = b_loc // F

    xe_ext = nc.declare_dram_parameter("xe", [128, b_loc], f32, isOutput=False)
    xic_ext = nc.declare_dram_parameter("xic", [40, b_loc], f32, isOutput=False)
    wR_ext = nc.declare_dram_parameter("wR", [128, wR_cols], f32, isOutput=False)
    wB_ext = nc.declare_dram_parameter("wB", [128, wB_cols], f32, isOutput=False)
    bb_ext = nc.declare_dram_parameter("bb", [128, bias_cols], f32, isOutput=False)

    o_pol = nc.declare_dram_parameter("pol", [16, b_loc], bf16, isOutput=True)
    o_val = nc.declare_dram_parameter("val", [1, b_loc], f32, isOutput=True)
    o_ign = nc.declare_dram_parameter("ign", [1, b_loc], f32, isOutput=True)
    o_s = nc.declare_dram_parameter("st", [1, b_loc], f32, isOutput=True)
    o_th = nc.declare_dram_parameter("th", [1, b_loc], f32, isOutput=True)
    o_pi = nc.declare_dram_parameter("pi2", [2, b_loc], f32, isOutput=True)
    o_som = nc.declare_dram_parameter("som", [64, b_loc], bf16, isOutput=True)
    o_gat = nc.declare_dram_parameter("gat", [64, b_loc], bf16, isOutput=True)

    return (xe_ext, xic_ext, wR_ext, wB_ext, bb_ext,
            o_pol, o_val, o_ign, o_s, o_th, o_pi, o_som, o_gat), nit


def emit(nc, tc, ext, nit, rs, bs, bbs, alpha, wR_cols, wB_cols, bias_cols):
    (xe_ext, xic_ext, wR_ext, wB_ext, bb_ext,
     o_pol, o_val, o_ign, o_s, o_th, o_pi, o_som, o_gat) = ext

    import contextlib
    ctx = contextlib.ExitStack()
    wpool = ctx.enter_context(tc.tile_pool(name="w", bufs=1))
    inp = ctx.enter_context(tc.tile_pool(name="inp", bufs=2))
    act = ctx.enter_context(tc.tile_pool(name="act", bufs=1))
    tl = ctx.enter_context(tc.tile_pool(name="tl", bufs=1))
    psw = ctx.enter_context(tc.tile_pool(name="psw", bufs=3, space="PSUM"))
    psh = ctx.enter_context(tc.tile_pool(name="psh", bufs=2, space="PSUM"))
    psp = ctx.enter_context(tc.tile_pool(name="psp", bufs=2, space="PSUM"))

    # --- weights / biases, loaded once ---
    wR = wpool.tile([128, wR_cols], f32r, name="wR")
    nc.gpsimd.dma_start(wR[:], wR_ext[:])
    wBt = wpool.tile([128, wB_cols], bf16, name="wBt")
    nc.gpsimd.dma_start(wBt[:], wB_ext[:])
    bb = wpool.tile([128, bias_cols], f32, name="bb")
    nc.sync.dma_start(bb[:], bb_ext[:])

    def W(name):
        r, c = bs[name]
        return wBt[r, c]

    def Bc(name, rows=slice(0, 128)):
        return bb[rows, bbs[name][0]:bbs[name][0] + 1]

    for it in range(nit):
        col0 = it * F
        xe = inp.tile([128, F], f32r, name="xe")
        nc.gpsimd.dma_start(xe[:], xe_ext[:, col0:col0 + F])
        xic = inp.tile([40, F], f32r, name="xic")
        nc.gpsimd.dma_start(xic[:], xic_ext[:, col0:col0 + F])

        h1e = act.tile([128, F], bf16, name="h1e", bufs=2)
        h1i = act.tile([64, F], bf16, name="h1i")
        th1h = tl.tile([128, C], bf16, name="th1h", bufs=2)
        h2 = act.tile([96, F], bf16, name="h2")
        c48 = act.tile([48, F], bf16, name="c48", bufs=2)
        c48sq = act.tile([48, F], bf16, name="c48sq")
        ws = act.tile([64, F], f32, name="ws", bufs=2)
        pepih = tl.tile([128, C], bf16, name="pepih", bufs=2)

        p_eps = psp.tile([128, C], f32, name="p_eps", tag="pp")
        p_pi = psp.tile([128, C], f32, name="p_pi", tag="pp")
        p_th = psp.tile([128, C], f32, name="p_th", tag="pp")

        for g in range(4):
            cs = slice(g * C, (g + 1) * C)
            rg = 32 * g
            # ---- encoders ----
            p1 = psw.tile([128, C], f32, name="p1", tag="pw")
            nc.tensor.matmul(p1[:], wR[rs["e1"][0], rs["e1"][1]], xe[:, cs],
                             start=True, stop=True)
            nc.scalar.activation(h1e[:, cs], p1[:], AF.Relu, bias=Bc("e1"))

            p2 = psw.tile([80, C], f32, name="p2", tag="pw")
            nc.tensor.matmul(p2[:], wR[rs["i1th1"][0], rs["i1th1"][1]],
                             xic[:, cs], start=True, stop=True)
            nc.scalar.activation(h1i[:, cs], p2[0:64, :], AF.Relu,
                                 bias=Bc("i1th1", slice(0, 64)))
            nc.scalar.activation(th1h[rg:rg + 16, :], p2[64:80, :], AF.Relu,
                                 bias=Bc("i1th1", slice(64, 80)))

            p3 = psw.tile([64, C], f32, name="p3", tag="pw")
            nc.tensor.matmul(p3[:], W("e2"), h1e[:, cs], start=True, stop=True)
            nc.scalar.activation(h2[0:64, cs], p3[:], AF.Relu,
                                 bias=Bc("e2", slice(0, 64)))

            p4 = psw.tile([32, C], f32, name="p4", tag="pw")
            nc.tensor.matmul(p4[:], W("i2"), h1i[:, cs], start=True, stop=True)
            nc.scalar.activation(h2[64:96, cs], p4[:], AF.Relu,
                                 bias=Bc("i2", slice(0, 32)))

            p5 = psw.tile([48, C], f32, name="p5", tag="pw")
            nc.tensor.matmul(p5[:], W("e3i3"), h2[:, cs], start=True, stop=True)
            nc.scalar.activation(c48[:, cs], p5[:], AF.Identity, bias=Bc("c48", slice(0, 48)))
            nc.scalar.activation(c48sq[:, cs], p5[:], AF.Square, bias=Bc("c48", slice(0, 48)))

            p6 = psw.tile([88, C], f32, name="p6", tag="pw")
            nc.tensor.matmul(p6[:], W("wspepi"), c48[:, cs], start=True, stop=True)
            nc.vector.tensor_scalar(ws[:, cs], p6[0:64, :],
                                    Bc("wspepi", slice(0, 64)), None, ALU.add)
            nc.scalar.activation(pepih[rg:rg + 24, :], p6[64:88, :], AF.Relu,
                                 bias=Bc("wspepi", slice(64, 88)))

        # ---- packed reductions (bf16, tile_position), grouped to limit
        # PE mode switches ----
        for g in range(4):
            cs = slice(g * C, (g + 1) * C)
            rg = 32 * g
            nc.tensor.matmul(p_eps[rg:rg + 2, :], W("epsones")[0:48, :],
                             c48sq[:, cs], start=True, stop=True,
                             tile_position=(0, rg))
            nc.tensor.matmul(p_pi[rg:rg + 2, :], W("pepi2")[rg:rg + 24, :],
                             pepih[rg:rg + 24, :], start=True, stop=True,
                             tile_position=(rg, rg))
            nc.tensor.matmul(p_th[rg:rg + 1, :], W("th2")[rg:rg + 16, :],
                             th1h[rg:rg + 16, :], start=True, stop=True,
                             tile_position=(rg, rg))

        # ---- narrow tail, packed x4 [128, C] ----
        t_ln = tl.tile([128, C], f32, name="t_ln", tag="sc", bufs=4)
        nc.scalar.activation(t_ln[:], p_eps[:], AF.Ln)
        t_eps = tl.tile([128, C], f32, name="t_eps")
        nc.scalar.activation(t_eps[:], t_ln[:], AF.Exp, bias=Bc("eps"), scale=0.5)

        t_pe = tl.tile([128, C], f32, name="t_pe", tag="sc", bufs=4)
        nc.scalar.activation(t_pe[:], p_pi[:], AF.Exp, bias=Bc("pi"))
        t_pi = tl.tile([128, C], f32, name="t_pi", bufs=2)
        nc.scalar.activation(t_pi[:], t_pe[:], AF.Ln, bias=1.0)

        t_sur = tl.tile([128, C], bf16, name="t_sur", bufs=2)
        nc.vector.tensor_mul(t_sur[:], t_eps[:], t_pi[:])

        # theta = sigmoid(th2+b) = recip(1+exp(-x-b))
        t_te = tl.tile([128, C], f32, name="t_te", tag="sc", bufs=4)
        nc.scalar.activation(t_te[:], p_th[:], AF.Exp, bias=Bc("thn"), scale=-1.0)
        nc.vector.tensor_scalar(t_te[:], t_te[:], 1.0, None, ALU.add)
        t_th = tl.tile([128, C], f32, name="t_th", bufs=2)
        nc.vector.reciprocal_approx_fast(t_th[:], t_te[:])

        # GRU gates
        p_r = psp.tile([128, C], f32, name="p_r", tag="pp")
        p_z = psp.tile([128, C], f32, name="p_z", tag="pp")
        p_n = psp.tile([128, C], f32, name="p_n", tag="pp")
        for g in range(4):
            rg = 32 * g
            nc.tensor.matmul(p_r[rg:rg + 16, :], W("gir")[rg:rg + 2, :],
                             t_sur[rg:rg + 2, :], start=True, stop=True,
                             tile_position=(rg, rg))
            nc.tensor.matmul(p_z[rg:rg + 16, :], W("giz")[rg:rg + 2, :],
                             t_sur[rg:rg + 2, :], start=True, stop=True,
                             tile_position=(rg, rg))
            nc.tensor.matmul(p_n[rg:rg + 16, :], W("gin")[rg:rg + 2, :],
                             t_sur[rg:rg + 2, :], start=True, stop=True,
                             tile_position=(rg, rg))

        # r = sigmoid(gi_r + br); rp = r*bhh_n
        t_er = tl.tile([128, C], f32, name="t_er", tag="sc", bufs=4)
        nc.scalar.activation(t_er[:], p_r[:], AF.Exp, bias=Bc("rn"), scale=-1.0)
        nc.vector.tensor_scalar(t_er[:], t_er[:], 1.0, None, ALU.add)
        t_r = tl.tile([128, C], f32, name="t_r")
        nc.vector.reciprocal_approx_fast(t_r[:], t_er[:])


        # zc = 1-z = recip(1+exp(+x+bz))
        t_ez = tl.tile([128, C], f32, name="t_ez", tag="sc", bufs=4)
        nc.scalar.activation(t_ez[:], p_z[:], AF.Exp, bias=Bc("z"))
        nc.vector.tensor_scalar(t_ez[:], t_ez[:], 1.0, None, ALU.add)
        t_zc = tl.tile([128, C], f32, name="t_zc")
        nc.vector.reciprocal_approx_fast(t_zc[:], t_ez[:])

        # n = tanh(gi_n + bn + r*bhh_n) = 1 - 2*recip(1+exp(2t+2bn))
        t_t = tl.tile([128, C], f32, name="t_t")
        nc.vector.scalar_tensor_tensor(t_t[:], t_r[:], Bc("bhhn"), p_n[:],
                                       ALU.mult, ALU.add)
        t_en = tl.tile([128, C], f32, name="t_en", tag="sc", bufs=4)
        nc.scalar.activation(t_en[:], t_t[:], AF.Exp, bias=Bc("n2"), scale=2.0)
        nc.vector.tensor_scalar(t_en[:], t_en[:], 1.0, None, ALU.add)
        t_w = tl.tile([128, C], f32, name="t_w")
        nc.vector.reciprocal_approx_fast(t_w[:], t_en[:])
        # h = zc*n = zc - 2*zc*w
        t_zw = tl.tile([128, C], f32, name="t_zw")
        nc.vector.scalar_tensor_tensor(t_zw[:], t_w[:], -2.0, t_zc[:],
                                       ALU.mult, ALU.mult)
        t_h = tl.tile([128, C], f32, name="t_h")
        nc.vector.tensor_add(t_h[:], t_zc[:], t_zw[:])
        t_h2 = tl.tile([128, C], bf16, name="t_h2")
        nc.scalar.activation(t_h2[:], t_h[:], AF.Square)

        p_s = psp.tile([128, C], f32, name="p_s", tag="pp")
        for g in range(4):
            rg = 32 * g
            nc.tensor.matmul(p_s[rg:rg + 1, :], W("ones16")[rg:rg + 16, :],
                             t_h2[rg:rg + 16, :], start=True, stop=True,
                             tile_position=(rg, rg))
        t_ls = tl.tile([128, C], f32, name="t_ls", tag="sc", bufs=4)
        nc.scalar.activation(t_ls[:], p_s[:], AF.Ln)
        t_s = tl.tile([128, C], f32, name="t_s", bufs=2)
        nc.scalar.activation(t_s[:], t_ls[:], AF.Exp, scale=0.5)

        # ignition = sigmoid(alpha*(S - theta))
        t_d = tl.tile([128, C], f32, name="t_d")
        nc.vector.tensor_sub(t_d[:], t_s[:], t_th[:])
        t_ei = tl.tile([128, C], f32, name="t_ei", tag="sc", bufs=4)
        nc.scalar.activation(t_ei[:], t_d[:], AF.Exp, scale=-alpha)
        nc.vector.tensor_scalar(t_ei[:], t_ei[:], 1.0, None, ALU.add)
        t_ign = tl.tile([128, C], f32, name="t_ign", bufs=2)
        nc.vector.reciprocal_approx_fast(t_ign[:], t_ei[:])
        ignb = tl.tile([128, C], bf16, name="ignb")
        nc.vector.tensor_copy(ignb[:], t_ign[:])

        # ---- heads ----
        gat = act.tile([64, F], bf16, name="gat", bufs=2)
        som = act.tile([64, F], bf16, name="som", bufs=2)
        so1h = act.tile([64, F], bf16, name="so1h")
        pv = act.tile([64, F], bf16, name="pv")
        expp = tl.tile([128, C], bf16, name="expp", bufs=2)
        pol = tl.tile([128, C], bf16, name="pol", bufs=2)
        valp = tl.tile([128, C], f32, name="valp", bufs=2)
        mod = act.tile([64, F], f32, name="mod")
        pin = act.tile([64, F], bf16, name="pin")

        p_den = psp.tile([128, C], f32, name="p_den", tag="pp")

        for g in range(4):
            cs = slice(g * C, (g + 1) * C)
            rg = 32 * g
            pg = psh.tile([64, C], f32, name="pg", tag="pg", bufs=1)
            nc.tensor.matmul(pg[:], W("ones64")[rg:rg + 1, :],
                             ignb[rg:rg + 1, :], start=True, stop=True,
                             tile_position=(rg, 0))
            nc.vector.tensor_mul(gat[:, cs], pg[:], ws[:, cs])

            p7 = psh.tile([96, C], f32, name="p7", tag="pw")
            nc.tensor.matmul(p7[:], W("so1va1"), gat[:, cs], start=True, stop=True)
            nc.vector.tensor_scalar(so1h[:, cs], p7[0:64, :],
                                    Bc("so1va1", slice(0, 64)), 0.0, ALU.add, ALU.max)
            nc.vector.tensor_scalar(pv[32:64, cs], p7[64:96, :],
                                    Bc("so1va1", slice(64, 96)), 0.0, ALU.add, ALU.max)

            p8 = psh.tile([64, C], f32, name="p8", tag="pw")
            nc.tensor.matmul(p8[:], W("so2"), so1h[:, cs], start=True, stop=True)
            nc.scalar.activation(som[:, cs], p8[:], AF.Identity, bias=Bc("so2", slice(0, 64)))
            # sigmoid(somatic): exp(-p8 - b) -> +1 -> recip
            nc.scalar.activation(mod[:, cs], p8[:], AF.Exp,
                                 bias=Bc("so2n", slice(0, 64)), scale=-1.0)
            nc.vector.tensor_scalar(mod[:, cs], mod[:, cs], 1.0, None, ALU.add)
            nc.vector.reciprocal_approx_fast(mod[:, cs], mod[:, cs])
            # pin = gat * (1 + 0.3*sig)
            nc.vector.tensor_scalar(mod[:, cs], mod[:, cs], 0.3, 1.0, ALU.mult, ALU.add)
            nc.vector.tensor_mul(pin[:, cs], gat[:, cs], mod[:, cs])

            p9 = psh.tile([32, C], f32, name="p9", tag="pw")
            nc.tensor.matmul(p9[:], W("po1"), pin[:, cs], start=True, stop=True)
            nc.vector.tensor_scalar(pv[0:32, cs], p9[:], Bc("po1", slice(0, 32)),
                                    0.0, ALU.add, ALU.max)

            p10 = psh.tile([33, C], f32, name="p10", tag="pw")
            nc.tensor.matmul(p10[:], W("po2va2"), pv[:, cs], start=True, stop=True)
            nc.scalar.activation(expp[rg:rg + 16, :], p10[0:16, :], AF.Exp,
                                 bias=Bc("po2", slice(0, 16)))
            nc.vector.tensor_scalar(valp[rg:rg + 1, :], p10[32:33, :],
                                    Bc("va2", slice(32, 33)), None, ALU.add)

            nc.tensor.matmul(p_den[rg:rg + 1, :], W("ones16")[rg:rg + 16, :],
                             expp[rg:rg + 16, :], start=True, stop=True,
                             tile_position=(rg, rg))

        t_rd = tl.tile([128, C], f32, name="t_rd")
        nc.vector.reciprocal_approx_fast(t_rd[:], p_den[:])
        t_rdb = tl.tile([128, C], bf16, name="t_rdb")
        nc.vector.tensor_copy(t_rdb[:], t_rd[:])
        for g in range(4):
            rg = 32 * g
            pdb = psh.tile([16, C], f32, name="pdb", tag="pw")
            nc.tensor.matmul(pdb[:], W("ones64")[rg:rg + 1, 0:16],
                             t_rdb[rg:rg + 1, :], start=True, stop=True,
                             tile_position=(rg, 0))
            nc.vector.tensor_mul(pol[rg:rg + 16, :], pdb[:], expp[rg:rg + 16, :])

        # ---- output DMAs ----
        nc.sync.dma_start(o_som[:, col0:col0 + F], som[:])
        nc.sync.dma_start(o_gat[:, col0:col0 + F], gat[:])
        for g in range(4):
            cs = slice(col0 + g * C, col0 + (g + 1) * C)
            rg = 32 * g
            nc.sync.dma_start(o_pol[:, cs], pol[rg:rg + 16, :])
            nc.sync.dma_start(o_val[:, cs], valp[rg:rg + 1, :])
            nc.sync.dma_start(o_ign[:, cs], t_ign[rg:rg + 1, :])
            nc.sync.dma_start(o_s[:, cs], t_s[rg:rg + 1, :])
            nc.sync.dma_start(o_th[:, cs], t_th[rg:rg + 1, :])
            nc.sync.dma_start(o_pi[:, cs], t_pi[rg:rg + 2, :])

    ctx.close()


_CACHED = {}


def _get_compiled(b_loc, wR_cols, wB_cols, bias_cols, rs, bs, bbs, alpha):
    key = (b_loc, wR_cols, wB_cols, bias_cols)
    if key in _CACHED:
        return _CACHED[key]
    nc = bacc.Bacc()
    ext, nit = build(nc, b_loc, wR_cols, wB_cols, bias_cols, alpha)
    with tile.TileContext(nc) as tc:
        emit(nc, tc, ext, nit, rs, bs, bbs, alpha, wR_cols, wB_cols, bias_cols)
    nc.compile()
    _CACHED[key] = nc
    return nc


def kernel(extero_input, intero_input, context, params, _b_loc=None, _trace=False):
    extero_input = np.asarray(extero_input, np.float32)
    intero_input = np.asarray(intero_input, np.float32)
    context = np.asarray(context, np.float32)
    b = extero_input.shape[0]
    b_loc = _b_loc or b // N_CORES

    wR, rs, wB, bs, bias, bbs, alpha = prep_weights(params)
    nc = _get_compiled(b_loc, wR.shape[1], wB.shape[1], bias.shape[1],
                       rs, bs, bbs, alpha)

    in_maps = []
    for cid in range(N_CORES):
        sl = slice(cid * b_loc, (cid + 1) * b_loc)
        xe = np.ascontiguousarray(extero_input[sl].T)
        xic = np.empty((40, b_loc), np.float32)
        xic[0:32] = intero_input[sl].T
        xic[32:40] = context[sl].T
        in_maps.append({"xe": xe, "xic": xic, "wR": wR, "wB": wB, "bb": bias})

    res = run_bass_kernel_spmd(nc, in_maps, core_ids=list(range(N_CORES)),
                               trace=_trace)
    outs = res.results

    def gather(name):
        return np.concatenate([outs[c][name] for c in range(N_CORES)], axis=1)

    pol = np.ascontiguousarray(gather("pol").T.astype(np.float32))
    val = np.ascontiguousarray(gather("val").T)
    ign = np.ascontiguousarray(gather("ign").T)
    st = np.ascontiguousarray(gather("st").T)
    th = np.ascontiguousarray(gather("th").T)
    pi = gather("pi2")
    pi_e = np.ascontiguousarray(pi[0:1].T)
    pi_i = np.ascontiguousarray(pi[1:2].T)
    som = np.ascontiguousarray(gather("som").T.astype(np.float32))
    gat = np.ascontiguousarray(gather("gat").T.astype(np.float32))
    if _trace:
        kernel._last_exec_time_ns = res.exec_time_ns
        kernel._last_res = res
    return pol, val, ign, st, th, pi_e, pi_i, som, gat


# revision 14
# speedup vs baseline: 2.1299x; 1.1791x over previous
"""APGI network Bass kernel for 8 TRN2 NeuronCores (pure data parallel).

Layout: feature-major (batch on the free/column axis). Host pre-transposes
inputs per core; device computes all 9 outputs feature-major; host
transposes back.

Self-contained: hardcodes shapes from the problem spec (B=524288, E=128,
I=32, C=8, A=16), 8 cores.
"""
import os
import sys
import numpy as np

sys.path.insert(0, "/opt/trn_rl_repo")

import concourse.bass as bass
import concourse.bacc as bacc
import concourse.tile as tile
from concourse import mybir
from concourse.bass_utils import run_bass_kernel_spmd

# Pin the ACT table to the single set covering every func we use
# (exp, ln, relu, identity, square) so bacc inserts ONE table load
# instead of ping-ponging between exp_and_others and natural_log.
from concourse import hw_specs as _hw_specs
_orig_gat = _hw_specs.get_activation_tables

def _single_set_tables(arch):
    d = _orig_gat(arch)
    return {k: (v if k == "natural_log_exp_and_others" else set())
            for k, v in d.items()}

bacc.get_activation_tables = _single_set_tables

AF = mybir.ActivationFunctionType
ALU = mybir.AluOpType
f32 = mybir.dt.float32
f32r = mybir.dt.float32r
bf16 = mybir.dt.bfloat16

B = 524288
N_CORES = 8
B_LOC = B // N_CORES

# Column tiling: F batch columns per iteration, split into 4 chunks of C.
F = 2048
C = F // 4


# ---------------------------------------------------------------------------
# Host-side weight packing
# ---------------------------------------------------------------------------
def _rep4(vals, offs, width=1):
    """Replicate a per-row pattern into all four 32-row blocks.

    vals: [n, width]; placed at rows 32g+offs .. 32g+offs+n for g in 0..3.
    """
    out = np.zeros((128, width), np.float32)
    v = np.asarray(vals, np.float32).reshape(-1, width)
    n = v.shape[0]
    for g in range(4):
        out[32 * g + offs:32 * g + offs + n, :] = v
    return out


def prep_weights(p):
    """Pack params dict into wR (f32r lhsT blob), wB (bf16 lhsT blob),
    bias blob, plus slice metadata. All np.float32 host-side."""
    P = {k: np.asarray(v, np.float32) for k, v in p.items()}
    alpha = float(abs(np.float32(P["alpha"])))
    beta = float(abs(np.float32(P["beta"])))

    # --- f32r blob: e1 and i1th1 (input layers) ---
    wR = np.zeros((128, 128 + 80), np.float32)
    wR[0:128, 0:128] = P["ew1"].T                     # e1 lhsT [128,128]
    wR[0:32, 128:192] = P["iw1"].T                    # i1 [32,64]
    wR[32:40, 192:208] = P["th_w1"].T                 # th1 [8,16]
    rs = {"e1": (slice(0, 128), slice(0, 128)),
          "i1th1": (slice(0, 40), slice(128, 208))}

    # --- bf16 blob ---
    cols = []
    bs = {}

    def add(name, rows, mat):
        off = sum(c.shape[1] for c in cols)
        m = np.zeros((128, mat.shape[1]), np.float32)
        m[rows, :] = mat
        cols.append(m)
        bs[name] = (rows, slice(off, off + mat.shape[1]))

    add("e2", slice(0, 128), P["ew2"].T)              # [128,64]
    add("i2", slice(0, 64), P["iw2"].T)               # [64,32]
    m = np.zeros((96, 48), np.float32)
    m[0:64, 0:32] = P["ew3"].T
    m[64:96, 32:48] = P["iw3"].T
    add("e3i3", slice(0, 96), m)
    m = np.zeros((48, 88), np.float32)
    m[:, 0:64] = P["ws_w"].T
    m[0:32, 64:80] = P["pe_w1"].T
    m[32:48, 80:88] = P["pi_w1"].T
    add("wspepi", slice(0, 48), m)
    m = np.zeros((64, 96), np.float32)
    m[:, 0:64] = P["so_w1"].T
    m[:, 64:96] = P["va_w1"].T
    add("so1va1", slice(0, 64), m)
    add("so2", slice(0, 64), P["so_w2"].T)
    add("po1", slice(0, 64), P["po_w1"].T)
    m = np.zeros((64, 33), np.float32)
    m[0:32, 0:16] = P["po_w2"].T
    m[32:64, 32:33] = P["va_w2"].T
    add("po2va2", slice(0, 64), m)
    # packed lhsTs
    m = np.zeros((48, 2), np.float32)
    m[0:32, 0] = 1.0
    m[32:48, 1] = 1.0
    add("epsones", slice(0, 48), m)                   # sum-sq reduce
    m = np.zeros((24, 2), np.float32)
    m[0:16, 0:1] = P["pe_w2"].T
    m[16:24, 1:2] = P["pi_w2"].T
    add("pepi2", slice(0, 128), _repmat(m, 24))
    add("th2", slice(0, 128), _repmat(P["th_w2"].T, 16))
    wih = P["gru_wih"]                                # [48,2]
    add("gir", slice(0, 128), _repmat(wih[0:16].T, 2))
    add("giz", slice(0, 128), _repmat(wih[16:32].T, 2))
    add("gin", slice(0, 128), _repmat(wih[32:48].T, 2))
    add("ones16", slice(0, 128), _repmat(np.ones((16, 1), np.float32), 16))
    add("ones64", slice(0, 128), np.ones((128, 64), np.float32))
    wB = np.concatenate(cols, axis=1)

    # --- bias blob (f32, one [128,1] column per entry) ---
    bcols = []
    bbs = {}

    def addb(name, col):
        bbs[name] = (sum(c.shape[1] for c in bcols),)
        bcols.append(np.asarray(col, np.float32).reshape(128, 1))

    def at(rows, vals):
        c = np.zeros((128, 1), np.float32)
        c[rows, 0] = vals
        return c

    addb("e1", at(slice(0, 128), P["eb1"]))
    c2 = np.zeros((128, 1), np.float32)
    c2[0:64, 0] = P["ib1"]
    c2[64:80, 0] = P["th_b1"]
    addb("i1th1", c2)
    addb("e2", at(slice(0, 64), P["eb2"]))
    addb("i2", at(slice(0, 32), P["ib2"]))
    c2 = np.zeros((128, 1), np.float32)
    c2[0:32, 0] = P["eb3"]
    c2[32:48, 0] = P["ib3"]
    addb("c48", c2)
    c2 = np.zeros((128, 1), np.float32)
    c2[0:64, 0] = P["ws_b"]
    c2[64:80, 0] = P["pe_b1"]
    c2[80:88, 0] = P["pi_b1"]
    addb("wspepi", c2)
    c2 = np.zeros((128, 1), np.float32)
    c2[0:64, 0] = P["so_b1"]
    c2[64:96, 0] = P["va_b1"]
    addb("so1va1", c2)
    addb("so2", at(slice(0, 64), P["so_b2"]))
    addb("so2n", at(slice(0, 64), -P["so_b2"]))
    addb("po1", at(slice(0, 32), P["po_b1"]))
    addb("po2", at(slice(0, 16), P["po_b2"]))
    addb("va2", at(slice(32, 33), P["va_b2"]))
    addb("eps", _rep4([[0.0], [np.log(beta)]], 0))
    addb("pi", _rep4([[P["pe_b2"][0]], [P["pi_b2"][0]]], 0))
    addb("thn", _rep4([[-P["th_b2"][0]]], 0))
    bih, bhh = P["gru_bih"], P["gru_bhh"]
    addb("rn", _rep4(-(bih[0:16] + bhh[0:16]).reshape(-1, 1), 0))
    addb("z", _rep4((bih[16:32] + bhh[16:32]).reshape(-1, 1), 0))
    addb("n2", _rep4((2.0 * bih[32:48]).reshape(-1, 1), 0))
    addb("bhhn", _rep4(bhh[32:48].reshape(-1, 1), 0))
    bias = np.concatenate(bcols, axis=1)

    return wR, rs, wB, bs, bias, bbs, alpha


def _repmat(m, nrows):
    """Replicate [nrows, w] matrix into rows 32g..32g+nrows for g=0..3."""
    out = np.zeros((128, m.shape[1]), np.float32)
    for g in range(4):
        out[32 * g:32 * g + nrows, :] = m
    return out


# ---------------------------------------------------------------------------
# Device kernel builder
# ---------------------------------------------------------------------------
def build(nc, b_loc, wR_cols, wB_cols, bias_cols, alpha):
    nit = b_loc // F

    xe_ext = nc.declare_dram_parameter("xe", [128, b_loc], f32, isOutput=False)
    xic_ext = nc.declare_dram_parameter("xic", [40, b_loc], f32, isOutput=False)
    wR_ext = nc.declare_dram_parameter("wR", [128, wR_cols], f32, isOutput=False)
    wB_ext = nc.declare_dram_parameter("wB", [128, wB_cols], f32, isOutput=False)
    bb_ext = nc.declare_dram_parameter("bb", [128, bias_cols], f32, isOutput=False)

    o_pol = nc.declare_dram_parameter("pol", [16, b_loc], bf16, isOutput=True)
    o_val = nc.declare_dram_parameter("val", [1, b_loc], f32, isOutput=True)
    o_ign = nc.declare_dram_parameter("ign", [1, b_loc], f32, isOutput=True)
    o_s = nc.declare_dram_parameter("st", [1, b_loc], f32, isOutput=True)
    o_th = nc.declare_dram_parameter("th", [1, b_loc], f32, isOutput=True)
    o_pi = nc.declare_dram_parameter("pi2", [2, b_loc], f32, isOutput=True)
    o_som = nc.declare_dram_parameter("som", [64, b_loc], bf16, isOutput=True)
    o_gat = nc.declare_dram_parameter("gat", [64, b_loc], bf16, isOutput=True)

    return (xe_ext, xic_ext, wR_ext, wB_ext, bb_ext,
            o_pol, o_val, o_ign, o_s, o_th, o_pi, o_som, o_gat), nit


def emit(nc, tc, ext, nit, rs, bs, bbs, alpha, wR_cols, wB_cols, bias_cols):
    (xe_ext, xic_ext, wR_ext, wB_ext, bb_ext,
     o_pol, o_val, o_ign, o_s, o_th, o_pi, o_som, o_gat) = ext

    import contextlib
    ctx = contextlib.ExitStack()
    wpool = ctx.enter_context(tc.tile_pool(name="w", bufs=1))
    inp = ctx.enter_context(tc.tile_pool(name="inp", bufs=2))
    act = ctx.enter_context(tc.tile_pool(name="act", bufs=1))
    tl = ctx.enter_context(tc.tile_pool(name="tl", bufs=1))
    psw = ctx.enter_context(tc.tile_pool(name="psw", bufs=2, space="PSUM"))
    psh = ctx.enter_context(tc.tile_pool(name="psh", bufs=2, space="PSUM"))
    psp = ctx.enter_context(tc.tile_pool(name="psp", bufs=3, space="PSUM"))

    # --- weights / biases, loaded once ---
    wR = wpool.tile([128, wR_cols], f32r, name="wR")
    nc.gpsimd.dma_start(wR[:], wR_ext[:])
    wBt = wpool.tile([128, wB_cols], bf16, name="wBt")
    nc.gpsimd.dma_start(wBt[:], wB_ext[:])
    bb = wpool.tile([128, bias_cols], f32, name="bb")
    nc.sync.dma_start(bb[:], bb_ext[:])

    def W(name):
        r, c = bs[name]
        return wBt[r, c]

    def Bc(name, rows=slice(0, 128)):
        return bb[rows, bbs[name][0]:bbs[name][0] + 1]

    for it in range(nit):
        col0 = it * F
        xe = inp.tile([128, F], f32r, name="xe")
        nc.gpsimd.dma_start(xe[:], xe_ext[:, col0:col0 + F])
        xic = inp.tile([40, F], f32r, name="xic")
        nc.gpsimd.dma_start(xic[:], xic_ext[:, col0:col0 + F])

        h1e = act.tile([128, F], bf16, name="h1e", bufs=2)
        h1i = act.tile([64, F], bf16, name="h1i")
        th1h = tl.tile([128, C], bf16, name="th1h", bufs=2)
        h2 = act.tile([96, F], bf16, name="h2")
        c48 = act.tile([48, F], bf16, name="c48", bufs=2)
        c48sq = act.tile([48, F], bf16, name="c48sq")
        ws = act.tile([64, F], f32, name="ws", bufs=2)
        pepih = tl.tile([128, C], bf16, name="pepih", bufs=2)

        p_eps = psp.tile([128, C], f32, name="p_eps", tag="pp")
        p_pi = psp.tile([128, C], f32, name="p_pi", tag="pp")
        p_th = psp.tile([128, C], f32, name="p_th", tag="pp")

        for g in range(4):
            cs = slice(g * C, (g + 1) * C)
            rg = 32 * g
            # ---- encoders ----
            p1 = psw.tile([128, C], f32, name="p1", tag="pw")
            nc.tensor.matmul(p1[:], wR[rs["e1"][0], rs["e1"][1]], xe[:, cs],
                             start=True, stop=True)
            nc.scalar.activation(h1e[:, cs], p1[:], AF.Relu, bias=Bc("e1"))

            p2 = psw.tile([80, C], f32, name="p2", tag="pw")
            nc.tensor.matmul(p2[:], wR[rs["i1th1"][0], rs["i1th1"][1]],
                             xic[:, cs], start=True, stop=True)
            nc.scalar.activation(h1i[:, cs], p2[0:64, :], AF.Relu,
                                 bias=Bc("i1th1", slice(0, 64)))
            nc.scalar.activation(th1h[rg:rg + 16, :], p2[64:80, :], AF.Relu,
                                 bias=Bc("i1th1", slice(64, 80)))

            p3 = psw.tile([64, C], f32, name="p3", tag="pw")
            nc.tensor.matmul(p3[:], W("e2"), h1e[:, cs], start=True, stop=True)
            nc.scalar.activation(h2[0:64, cs], p3[:], AF.Relu,
                                 bias=Bc("e2", slice(0, 64)))

            p4 = psw.tile([32, C], f32, name="p4", tag="pw")
            nc.tensor.matmul(p4[:], W("i2"), h1i[:, cs], start=True, stop=True)
            nc.scalar.activation(h2[64:96, cs], p4[:], AF.Relu,
                                 bias=Bc("i2", slice(0, 32)))

            p5 = psw.tile([48, C], f32, name="p5", tag="pw")
            nc.tensor.matmul(p5[:], W("e3i3"), h2[:, cs], start=True, stop=True)
            nc.scalar.activation(c48[:, cs], p5[:], AF.Identity, bias=Bc("c48", slice(0, 48)))
            nc.scalar.activation(c48sq[:, cs], p5[:], AF.Square, bias=Bc("c48", slice(0, 48)))

            p6 = psw.tile([88, C], f32, name="p6", tag="pw")
            nc.tensor.matmul(p6[:], W("wspepi"), c48[:, cs], start=True, stop=True)
            nc.vector.tensor_scalar(ws[:, cs], p6[0:64, :],
                                    Bc("wspepi", slice(0, 64)), None, ALU.add)
            nc.scalar.activation(pepih[rg:rg + 24, :], p6[64:88, :], AF.Relu,
                                 bias=Bc("wspepi", slice(64, 88)))

        # ---- packed reductions (bf16, tile_position), grouped to limit
        # PE mode switches ----
        for g in range(4):
            cs = slice(g * C, (g + 1) * C)
            rg = 32 * g
            nc.tensor.matmul(p_eps[rg:rg + 2, :], W("epsones")[0:48, :],
                             c48sq[:, cs], start=True, stop=True,
                             tile_position=(0, rg))
            nc.tensor.matmul(p_pi[rg:rg + 2, :], W("pepi2")[rg:rg + 24, :],
                             pepih[rg:rg + 24, :], start=True, stop=True,
                             tile_position=(rg, rg))
            nc.tensor.matmul(p_th[rg:rg + 1, :], W("th2")[rg:rg + 16, :],
                             th1h[rg:rg + 16, :], start=True, stop=True,
                             tile_position=(rg, rg))

        # ---- narrow tail, packed x4 [128, C] ----
        t_ln = tl.tile([128, C], f32, name="t_ln", tag="sc", bufs=4)
        nc.scalar.activation(t_ln[:], p_eps[:], AF.Ln)
        t_eps = tl.tile([128, C], f32, name="t_eps")
        nc.scalar.activation(t_eps[:], t_ln[:], AF.Exp, bias=Bc("eps"), scale=0.5)

        t_pe = tl.tile([128, C], f32, name="t_pe", tag="sc", bufs=4)
        nc.scalar.activation(t_pe[:], p_pi[:], AF.Exp, bias=Bc("pi"))
        t_pi = tl.tile([128, C], f32, name="t_pi", bufs=2)
        nc.scalar.activation(t_pi[:], t_pe[:], AF.Ln, bias=1.0)

        t_sur = tl.tile([128, C], bf16, name="t_sur", bufs=2)
        nc.vector.tensor_mul(t_sur[:], t_eps[:], t_pi[:])

        # theta = sigmoid(th2+b) = recip(1+exp(-x-b))
        t_te = tl.tile([128, C], f32, name="t_te", tag="sc", bufs=4)
        nc.scalar.activation(t_te[:], p_th[:], AF.Exp, bias=Bc("thn"), scale=-1.0)
        nc.vector.tensor_scalar(t_te[:], t_te[:], 1.0, None, ALU.add)
        t_th = tl.tile([128, C], f32, name="t_th", bufs=2)
        nc.vector.reciprocal_approx_fast(t_th[:], t_te[:])

        # GRU gates
        p_r = psp.tile([128, C], f32, name="p_r", tag="pp")
        p_z = psp.tile([128, C], f32, name="p_z", tag="pp")
        p_n = psp.tile([128, C], f32, name="p_n", tag="pp")
        for g in range(4):
            rg = 32 * g
            nc.tensor.matmul(p_r[rg:rg + 16, :], W("gir")[rg:rg + 2, :],
                             t_sur[rg:rg + 2, :], start=True, stop=True,
                             tile_position=(rg, rg))
            nc.tensor.matmul(p_z[rg:rg + 16, :], W("giz")[rg:rg + 2, :],
                             t_sur[rg:rg + 2, :], start=True, stop=True,
                             tile_position=(rg, rg))
            nc.tensor.matmul(p_n[rg:rg + 16, :], W("gin")[rg:rg + 2, :],
                             t_sur[rg:rg + 2, :], start=True, stop=True,
                             tile_position=(rg, rg))

        # r = sigmoid(gi_r + br); rp = r*bhh_n
        t_er = tl.tile([128, C], f32, name="t_er", tag="sc", bufs=4)
        nc.scalar.activation(t_er[:], p_r[:], AF.Exp, bias=Bc("rn"), scale=-1.0)
        nc.vector.tensor_scalar(t_er[:], t_er[:], 1.0, None, ALU.add)
        t_r = tl.tile([128, C], f32, name="t_r")
        nc.vector.reciprocal_approx_fast(t_r[:], t_er[:])


        # zc = 1-z = recip(1+exp(+x+bz))
        t_ez = tl.tile([128, C], f32, name="t_ez", tag="sc", bufs=4)
        nc.scalar.activation(t_ez[:], p_z[:], AF.Exp, bias=Bc("z"))
        nc.vector.tensor_scalar(t_ez[:], t_ez[:], 1.0, None, ALU.add)
        t_zc = tl.tile([128, C], f32, name="t_zc")
        nc.vector.reciprocal_approx_fast(t_zc[:], t_ez[:])

        # n = tanh(gi_n + bn + r*bhh_n) = 1 - 2*recip(1+exp(2t+2bn))
        t_t = tl.tile([128, C], f32, name="t_t")
        nc.vector.scalar_tensor_tensor(t_t[:], t_r[:], Bc("bhhn"), p_n[:],
                                       ALU.mult, ALU.add)
        t_en = tl.tile([128, C], f32, name="t_en", tag="sc", bufs=4)
        nc.scalar.activation(t_en[:], t_t[:], AF.Exp, bias=Bc("n2"), scale=2.0)
        nc.vector.tensor_scalar(t_en[:], t_en[:], 1.0, None, ALU.add)
        t_w = tl.tile([128, C], f32, name="t_w")
        nc.vector.reciprocal_approx_fast(t_w[:], t_en[:])
        # h = zc*n = zc - 2*zc*w
        t_zw = tl.tile([128, C], f32, name="t_zw")
        nc.vector.scalar_tensor_tensor(t_zw[:], t_w[:], -2.0, t_zc[:],
                                       ALU.mult, ALU.mult)
        t_h = tl.tile([128, C], f32, name="t_h")
        nc.vector.tensor_add(t_h[:], t_zc[:], t_zw[:])
        t_h2 = tl.tile([128, C], bf16, name="t_h2")
        nc.scalar.activation(t_h2[:], t_h[:], AF.Square)

        p_s = psp.tile([128, C], f32, name="p_s", tag="pp")
        for g in range(4):
            rg = 32 * g
            nc.tensor.matmul(p_s[rg:rg + 1, :], W("ones16")[rg:rg + 16, :],
                             t_h2[rg:rg + 16, :], start=True, stop=True,
                             tile_position=(rg, rg))
        t_ls = tl.tile([128, C], f32, name="t_ls", tag="sc", bufs=4)
        nc.scalar.activation(t_ls[:], p_s[:], AF.Ln)
        t_s = tl.tile([128, C], f32, name="t_s", bufs=2)
        nc.scalar.activation(t_s[:], t_ls[:], AF.Exp, scale=0.5)

        # ignition = sigmoid(alpha*(S - theta))
        t_d = tl.tile([128, C], f32, name="t_d")
        nc.vector.tensor_sub(t_d[:], t_s[:], t_th[:])
        t_ei = tl.tile([128, C], f32, name="t_ei", tag="sc", bufs=4)
        nc.scalar.activation(t_ei[:], t_d[:], AF.Exp, scale=-alpha)
        nc.vector.tensor_scalar(t_ei[:], t_ei[:], 1.0, None, ALU.add)
        t_ign = tl.tile([128, C], f32, name="t_ign", bufs=2)
        nc.vector.reciprocal_approx_fast(t_ign[:], t_ei[:])
        ignb = tl.tile([128, C], bf16, name="ignb")
        nc.vector.tensor_copy(ignb[:], t_ign[:])

        # ---- heads ----
        gat = act.tile([64, F], bf16, name="gat", bufs=2)
        som = act.tile([64, F], bf16, name="som", bufs=2)
        so1h = act.tile([64, F], bf16, name="so1h")
        pv = act.tile([64, F], bf16, name="pv")
        expp = tl.tile([128, C], bf16, name="expp", bufs=2)
        pol = tl.tile([128, C], bf16, name="pol", bufs=2)
        valp = tl.tile([128, C], f32, name="valp", bufs=2)
        mod = act.tile([64, F], f32, name="mod")
        pin = act.tile([64, F], bf16, name="pin")

        p_den = psp.tile([128, C], f32, name="p_den", tag="pp")

        for g in range(4):
            cs = slice(g * C, (g + 1) * C)
            rg = 32 * g
            pg = psh.tile([64, C], f32, name="pg", tag="pg", bufs=1)
            nc.tensor.matmul(pg[:], W("ones64")[rg:rg + 1, :],
                             ignb[rg:rg + 1, :], start=True, stop=True,
                             tile_position=(rg, 0))
            nc.vector.tensor_mul(gat[:, cs], pg[:], ws[:, cs])

            p7 = psh.tile([96, C], f32, name="p7", tag="pw")
            nc.tensor.matmul(p7[:], W("so1va1"), gat[:, cs], start=True, stop=True)
            nc.vector.tensor_scalar(so1h[:, cs], p7[0:64, :],
                                    Bc("so1va1", slice(0, 64)), 0.0, ALU.add, ALU.max)
            nc.vector.tensor_scalar(pv[32:64, cs], p7[64:96, :],
                                    Bc("so1va1", slice(64, 96)), 0.0, ALU.add, ALU.max)

            p8 = psh.tile([64, C], f32, name="p8", tag="pw")
            nc.tensor.matmul(p8[:], W("so2"), so1h[:, cs], start=True, stop=True)
            nc.scalar.activation(som[:, cs], p8[:], AF.Identity, bias=Bc("so2", slice(0, 64)))
            # sigmoid(somatic): exp(-p8 - b) -> +1 -> recip
            nc.scalar.activation(mod[:, cs], p8[:], AF.Exp,
                                 bias=Bc("so2n", slice(0, 64)), scale=-1.0)
            nc.vector.tensor_scalar(mod[:, cs], mod[:, cs], 1.0, None, ALU.add)
            nc.vector.reciprocal_approx_fast(mod[:, cs], mod[:, cs])
            # pin = gat * (1 + 0.3*sig)
            nc.vector.tensor_scalar(mod[:, cs], mod[:, cs], 0.3, 1.0, ALU.mult, ALU.add)
            nc.vector.tensor_mul(pin[:, cs], gat[:, cs], mod[:, cs])

            p9 = psh.tile([32, C], f32, name="p9", tag="pw")
            nc.tensor.matmul(p9[:], W("po1"), pin[:, cs], start=True, stop=True)
            nc.vector.tensor_scalar(pv[0:32, cs], p9[:], Bc("po1", slice(0, 32)),
                                    0.0, ALU.add, ALU.max)

            p10 = psh.tile([33, C], f32, name="p10", tag="pw")
            nc.tensor.matmul(p10[:], W("po2va2"), pv[:, cs], start=True, stop=True)
            nc.scalar.activation(expp[rg:rg + 16, :], p10[0:16, :], AF.Exp,
                                 bias=Bc("po2", slice(0, 16)))
            nc.vector.tensor_scalar(valp[rg:rg + 1, :], p10[32:33, :],
                                    Bc("va2", slice(32, 33)), None, ALU.add)

            nc.tensor.matmul(p_den[rg:rg + 1, :], W("ones16")[rg:rg + 16, :],
                             expp[rg:rg + 16, :], start=True, stop=True,
                             tile_position=(rg, rg))

        t_rd = tl.tile([128, C], f32, name="t_rd")
        nc.vector.reciprocal_approx_fast(t_rd[:], p_den[:])
        t_rdb = tl.tile([128, C], bf16, name="t_rdb")
        nc.vector.tensor_copy(t_rdb[:], t_rd[:])
        for g in range(4):
            rg = 32 * g
            pdb = psh.tile([16, C], f32, name="pdb", tag="pw")
            nc.tensor.matmul(pdb[:], W("ones64")[rg:rg + 1, 0:16],
                             t_rdb[rg:rg + 1, :], start=True, stop=True,
                             tile_position=(rg, 0))
            nc.vector.tensor_mul(pol[rg:rg + 16, :], pdb[:], expp[rg:rg + 16, :])

        # ---- output DMAs ----
        nc.sync.dma_start(o_som[:, col0:col0 + F], som[:])
        nc.sync.dma_start(o_gat[:, col0:col0 + F], gat[:])
        for g in range(4):
            cs = slice(col0 + g * C, col0 + (g + 1) * C)
            rg = 32 * g
            nc.sync.dma_start(o_pol[:, cs], pol[rg:rg + 16, :])
            nc.sync.dma_start(o_val[:, cs], valp[rg:rg + 1, :])
            nc.sync.dma_start(o_ign[:, cs], t_ign[rg:rg + 1, :])
            nc.sync.dma_start(o_s[:, cs], t_s[rg:rg + 1, :])
            nc.sync.dma_start(o_th[:, cs], t_th[rg:rg + 1, :])
            nc.sync.dma_start(o_pi[:, cs], t_pi[rg:rg + 2, :])

    ctx.close()


_CACHED = {}


def _get_compiled(b_loc, wR_cols, wB_cols, bias_cols, rs, bs, bbs, alpha):
    key = (b_loc, wR_cols, wB_cols, bias_cols)
    if key in _CACHED:
        return _CACHED[key]
    nc = bacc.Bacc()
    ext, nit = build(nc, b_loc, wR_cols, wB_cols, bias_cols, alpha)
    with tile.TileContext(nc) as tc:
        emit(nc, tc, ext, nit, rs, bs, bbs, alpha, wR_cols, wB_cols, bias_cols)
    nc.compile()
    _CACHED[key] = nc
    return nc


def kernel(extero_input, intero_input, context, params, _b_loc=None, _trace=False):
    extero_input = np.asarray(extero_input, np.float32)
    intero_input = np.asarray(intero_input, np.float32)
    context = np.asarray(context, np.float32)
    b = extero_input.shape[0]
    b_loc = _b_loc or b // N_CORES

    wR, rs, wB, bs, bias, bbs, alpha = prep_weights(params)
    nc = _get_compiled(b_loc, wR.shape[1], wB.shape[1], bias.shape[1],
                       rs, bs, bbs, alpha)

    in_maps = []
    for cid in range(N_CORES):
        sl = slice(cid * b_loc, (cid + 1) * b_loc)
        xe = np.ascontiguousarray(extero_input[sl].T)
        xic = np.empty((40, b_loc), np.float32)
        xic[0:32] = intero_input[sl].T
        xic[32:40] = context[sl].T
        in_maps.append({"xe": xe, "xic": xic, "wR": wR, "wB": wB, "bb": bias})

    res = run_bass_kernel_spmd(nc, in_maps, core_ids=list(range(N_CORES)),
                               trace=_trace)
    outs = res.results

    def gather(name):
        return np.concatenate([outs[c][name] for c in range(N_CORES)], axis=1)

    pol = np.ascontiguousarray(gather("pol").T.astype(np.float32))
    val = np.ascontiguousarray(gather("val").T)
    ign = np.ascontiguousarray(gather("ign").T)
    st = np.ascontiguousarray(gather("st").T)
    th = np.ascontiguousarray(gather("th").T)
    pi = gather("pi2")
    pi_e = np.ascontiguousarray(pi[0:1].T)
    pi_i = np.ascontiguousarray(pi[1:2].T)
    som = np.ascontiguousarray(gather("som").T.astype(np.float32))
    gat = np.ascontiguousarray(gather("gat").T.astype(np.float32))
    if _trace:
        kernel._last_exec_time_ns = res.exec_time_ns
        kernel._last_res = res
    return pol, val, ign, st, th, pi_e, pi_i, som, gat
